# revision 1
# baseline (speedup 1.0000x reference)
"""BertBlock (mean-only LN, 16-head attention, relu FF) on 8 trn2 NeuronCores.

Sharding: head-parallel attention (2 heads / core) + sequence-parallel
norms & FF (512 rows / core). Collectives: one AllGather of the normed
input (transposed layout) + one ReduceScatter after out-proj. FF runs
fully locally on each core's row shard with full (host-pre-transposed)
FF weights streamed from HBM, so no collective is needed after FF2.

All heavy matmuls run as float32r (fast fp32 mode, ~1e-4 rel err).
"""
import sys

sys.path.insert(0, '/opt/trn_rl_repo')

import numpy as np
import concourse.bass as bass
from concourse import bacc
import concourse.mybir as mybir
import concourse.tile as tile
from concourse.masks import make_identity

S = 4096          # sequence length
H = 1024          # hidden
I_ = 4096         # ffn inner
NH = 16           # heads
HD = 64           # head dim
NC = 8            # cores
SM = S // NC      # 512 rows per core
DM = 128          # inner dims per core (2 heads x 64)
HC = H // 128     # 8 hidden chunks
ST = S // 512     # 8 s-tiles of 512
F32 = mybir.dt.float32
F32R = mybir.dt.float32r
BF16 = mybir.dt.bfloat16
AF = mybir.ActivationFunctionType
ALU = mybir.AluOpType
AXX = mybir.AxisListType.X

_CACHE = {}


def build_nc():
    nc = bacc.Bacc(None, target_bir_lowering=False, debug=False)
    P = lambda name, shape: nc.declare_dram_parameter(name, shape, F32, isOutput=False)
    x_m = P("x_m", [SM, H])
    wqkvT = P("wqkvT", [H, 3 * DM])      # [h, q|k|v cols of my 2 heads]
    bqkv = P("bqkv", [1, 3 * DM])
    owT = P("owT", [DM, H])              # o_w[:, my_cols].T
    ob = P("ob", [1, H])
    anw, anb = P("anw", [1, H]), P("anb", [1, H])
    fnw, fnb = P("fnw", [1, H]), P("fnb", [1, H])
    ff1wT = P("ff1wT", [H, I_])
    ff1b = P("ff1b", [32, 128])
    ff2wT = P("ff2wT", [I_, H])
    ffb2 = P("ffb2", [1, H])
    y = nc.declare_dram_parameter("y", [SM, H], F32, isOutput=True)

    with tile.TileContext(nc) as tc:
        cst = tc.alloc_tile_pool(name="cst", bufs=1)
        dram = tc.alloc_tile_pool(name="dram", bufs=1, space="DRAM")
        xmp = tc.alloc_tile_pool(name="xmp", bufs=1)
        setp = tc.alloc_tile_pool(name="setp", bufs=1)
        ps_set = tc.alloc_tile_pool(name="ps_set", bufs=2, space="PSUM")

        ag_in = dram.tile([H, SM], F32)
        ag_out = dram.tile([NC, H, SM], F32, addr_space="Shared")
        rs_in = dram.tile([S, H], F32)
        rs_out = dram.tile([SM, H], F32)

        # ---- constants ----
        ident = cst.tile([128, 128], F32)
        make_identity(nc, ident)
        ones_f = cst.tile([1, 128], F32)
        nc.gpsimd.memset(ones_f, 1.0)
        ones1 = cst.tile([1, 128], F32R)
        nc.vector.tensor_copy(ones1[:], ones_f[:])
        ones_col = cst.tile([128, 1], F32)
        nc.gpsimd.memset(ones_col, 1.0)

        def load_vec(p):
            t = setp.tile([1, H], F32, tag=f"v_{p.name}")
            nc.sync.dma_start(out=t[:], in_=p[:])
            return t

        vecs = {n: load_vec(p) for n, p in
                [("anw", anw), ("anb", anb), ("fnw", fnw), ("fnb", fnb),
                 ("ob", ob), ("ffb2", ffb2)]}

        def bcast(name, pool=None):
            # [1, H] -> [128, H] broadcast across partitions via PE
            v = vecs[name]
            bc = (pool or cst).tile([128, H], F32, tag=f"bc_{name}", name=f"bc_{name}")
            for hf in range(H // 512):
                ps = ps_set.tile([128, 512], F32)
                nc.tensor.matmul(ps[:], ones_f[0:1, :], v[0:1, hf * 512:(hf + 1) * 512],
                                 start=True, stop=True)
                nc.vector.tensor_copy(bc[:, hf * 512:(hf + 1) * 512], ps[:])
            return bc

        anw_bc, anb_bc = bcast("anw", setp), bcast("anb", setp)
        fnw_bc, fnb_bc = bcast("fnw"), bcast("fnb")
        ob_bc, ffb2_bc = bcast("ob"), bcast("ffb2")

        bqkv_sb = setp.tile([1, 3 * DM], F32)
        nc.sync.dma_start(out=bqkv_sb[:], in_=bqkv[:])
        qkvb_pp = []
        for j in range(3):
            ps = ps_set.tile([128, 512], F32)
            nc.tensor.matmul(ps[:, 0:1],
                             bqkv_sb[0:1, j * 128:(j + 1) * 128],
                             ones_f[0:1, 0:1], start=True, stop=True)
            t = cst.tile([128, 1], F32, tag=f"b_pp{j}")
            nc.vector.tensor_copy(t[:], ps[:, 0:1])
            qkvb_pp.append(t)

        ffb1_ld = setp.tile([32, 128], F32)
        nc.sync.dma_start(out=ffb1_ld[:], in_=ff1b[:])
        ps = ps_set.tile([128, 512], F32)
        nc.tensor.transpose(ps[:, 0:32], ffb1_ld[:], ident[0:32, 0:32])
        ffb1_pp = cst.tile([128, 32], F32)
        nc.vector.tensor_copy(ffb1_pp[:], ps[:, 0:32])

        # ---- LN1 on my rows + transpose + AllGather ----
        xm_tiles = []
        for i in range(4):
            t = xmp.tile([128, H], F32, tag=f"xm{i}")
            nc.sync.dma_start(out=t[:], in_=x_m[i * 128:(i + 1) * 128, :])
            xm_tiles.append(t)

        with tc.tile_pool(name="ln1", bufs=1) as lnp, \
             tc.tile_pool(name="ln1s", bufs=3) as lnsp:
            xn_tiles = []
            for i in range(4):
                ns = lnsp.tile([128, 1], F32, tag="negsum")
                nc.vector.reduce_sum(out=ns[:], in_=xm_tiles[i][:], axis=AXX,
                                     negate=True)
                nm = lnsp.tile([128, 1], F32, tag="negmean")
                nc.scalar.mul(nm[:], ns[:], 1.0 / H)
                xn = lnp.tile([128, H], F32, tag=f"xn{i}")
                nc.vector.scalar_tensor_tensor(
                    out=xn[:], in0=xm_tiles[i][:], scalar=nm[:], in1=anw_bc[:],
                    op0=ALU.add, op1=ALU.mult)
                nc.vector.tensor_add(xn[:], xn[:], anb_bc[:])
                xn_tiles.append(xn)
            for hc in range(HC):
                xt = lnp.tile([128, SM], F32, tag=f"xnT{hc}")
                for si in range(4):
                    ps = ps_set.tile([128, 512], F32, tag="tps")
                    nc.tensor.transpose(ps[:, 0:128],
                                        xn_tiles[si][:, hc * 128:(hc + 1) * 128],
                                        ident[:])
                    nc.vector.tensor_copy(xt[:, si * 128:(si + 1) * 128],
                                          ps[:, 0:128])
                nc.sync.dma_start(out=ag_in[hc * 128:(hc + 1) * 128, :], in_=xt[:])
            nc.gpsimd.collective_compute(
                "AllGather", ALU.bypass, replica_groups=[list(range(NC))],
                ins=[ag_in.opt()], outs=[ag_out.opt()])
        ps_set.release()
        setp.release()

        # ---- attention ----
        with tc.tile_pool(name="attn", bufs=1) as at, \
             tc.tile_pool(name="stream", bufs=10) as stp, \
             tc.tile_pool(name="vtp", bufs=3) as vtp, \
             tc.tile_pool(name="expp", bufs=6) as expp, \
             tc.tile_pool(name="rcp", bufs=2) as rcp, \
             tc.tile_pool(name="aop", bufs=3) as aop, \
             tc.tile_pool(name="ps_mm", bufs=3, space="PSUM") as ps_mm, \
             tc.tile_pool(name="ps_acc", bufs=1, space="PSUM") as ps_acc, \
             tc.tile_pool(name="ps_bc", bufs=1, space="PSUM") as ps_bc:

            wqkv_t = []
            for hc in range(HC):
                t = at.tile([128, 3 * DM], F32R, tag=f"wqkv{hc}")
                nc.sync.dma_start(out=t[:],
                                  in_=wqkvT[hc * 128:(hc + 1) * 128, :].bitcast(F32R))
                wqkv_t.append(t)
            owT_sb = at.tile([DM, H], F32R, tag="owT")
            nc.sync.dma_start(out=owT_sb[:], in_=owT[:].bitcast(F32R))

            QTp = [at.tile([128, S], BF16, tag=f"QTp{h}", name=f"QTp{h}")
                   for h in range(2)]
            KTp = [at.tile([128, S], BF16, tag=f"KTp{h}", name=f"KTp{h}")
                   for h in range(2)]
            for h in range(2):
                z = slice(HD, 128) if h == 0 else slice(0, HD)
                nc.gpsimd.memset(QTp[h][z, :], 0.0)
                nc.gpsimd.memset(KTp[h][z, :], 0.0)
            vaug = [[at.tile([128, HD + 1], BF16, tag=f"va{h}_{t}", name=f"va{h}_{t}")
                     for t in range(32)] for h in range(2)]
            for h in range(2):
                for t in range(32):
                    nc.vector.tensor_copy(vaug[h][t][:, HD:HD + 1], ones_col[:])

            for r in range(ST):
                xnr = []
                for hc in range(HC):
                    t = stp.tile([128, 512], F32R, tag="xnr")
                    nc.sync.dma_start(
                        out=t[:], in_=ag_out[r, hc * 128:(hc + 1) * 128, :].bitcast(F32R))
                    xnr.append(t)
                for j, dest in ((0, QTp), (1, KTp)):
                    ps = ps_mm.tile([128, 512], F32, tag="mm")
                    for hc in range(HC):
                        nc.tensor.matmul(ps[:], wqkv_t[hc][:, j * 128:(j + 1) * 128],
                                         xnr[hc][:], start=(hc == 0), stop=(hc == 7))
                    for h in range(2):
                        hs = slice(h * HD, (h + 1) * HD)
                        nc.vector.tensor_scalar_add(
                            dest[h][hs, r * 512:(r + 1) * 512], ps[hs, :],
                            qkvb_pp[j][hs, :])
                ps = ps_mm.tile([128, 512], F32, tag="mm")
                for hc in range(HC):
                    nc.tensor.matmul(ps[:], wqkv_t[hc][:, 2 * 128:3 * 128],
                                     xnr[hc][:], start=(hc == 0), stop=(hc == 7))
                vtmp = vtp.tile([128, 512], F32, tag="vtmp")
                nc.vector.tensor_scalar_add(vtmp[:], ps[:], qkvb_pp[2][:])
                for tb in range(4):
                    pst = ps_bc.tile([128, 128], F32, tag="vtp")
                    nc.tensor.transpose(pst[:], vtmp[:, tb * 128:(tb + 1) * 128],
                                        ident[:])
                    ti = r * 4 + tb
                    nc.vector.tensor_copy(vaug[0][ti][:, 0:HD], pst[:, 0:HD])
                    nc.vector.tensor_copy(vaug[1][ti][:, 0:HD], pst[:, HD:2 * HD])

            ctxT = at.tile([128, S], F32R, tag="ctxT")
            chains = [(h, r) for h in range(2) for r in range(ST)]
            for g in range(0, len(chains), 3):
                grp = chains[g:g + 3]
                cps_l = [ps_acc.tile([128, 512], F32, name=f"cps{g}_{i}",
                                     tag=f"cps{i}")
                         for i in range(len(grp))]
                for t in range(32):
                    exl = []
                    for (h, r), cps in zip(grp, cps_l):
                        sps = ps_mm.tile([128, 512], F32, tag="mm")
                        nc.tensor.matmul(sps[:],
                                         KTp[h][:, t * 128:(t + 1) * 128],
                                         QTp[h][:, r * 512:(r + 1) * 512],
                                         start=True, stop=True)
                        ex = expp.tile([128, 512], BF16, tag="exp")
                        nc.scalar.activation(ex[:], sps[:], AF.Exp, scale=0.125)
                        exl.append(ex)
                    for (h, r), cps, ex in zip(grp, cps_l, exl):
                        nc.tensor.matmul(cps[0:HD + 1, :], vaug[h][t][:, :],
                                         ex[:], start=(t == 0), stop=(t == 31),
                                         skip_group_check=True)
                for (h, r), cps in zip(grp, cps_l):
                    hs = slice(h * HD, (h + 1) * HD)
                    rc = rcp.tile([1, 512], F32R, tag="rc")
                    with nc.allow_low_precision(reason="f32r softmax denom"):
                        nc.vector.reciprocal(rc[:], cps[HD:HD + 1, :])
                    bps = ps_bc.tile([128, 512], F32, tag="rbc")
                    nc.tensor.matmul(bps[0:HD, :], ones1[0:1, 0:HD], rc[0:1, :],
                                     start=True, stop=True)
                    bsb = rcp.tile([HD, 512], F32, tag="bsb")
                    nc.vector.tensor_copy(bsb[:], bps[0:HD, :])
                    nc.vector.tensor_mul(ctxT[hs, r * 512:(r + 1) * 512],
                                         cps[0:HD, :], bsb[:])

            for s128 in range(32):
                ao = aop.tile([128, H], F32, tag="ao")
                for hf in range(2):
                    ps = ps_mm.tile([128, 512], F32, tag="mm")
                    nc.tensor.matmul(ps[:], ctxT[:, s128 * 128:(s128 + 1) * 128],
                                     owT_sb[:, hf * 512:(hf + 1) * 512],
                                     start=True, stop=True)
                    nc.vector.tensor_copy(ao[:, hf * 512:(hf + 1) * 512], ps[:])
                nc.sync.dma_start(out=rs_in[s128 * 128:(s128 + 1) * 128, :], in_=ao[:])

        nc.gpsimd.collective_compute(
            "ReduceScatter", ALU.add, replica_groups=[list(range(NC))],
            ins=[rs_in.opt()], outs=[rs_out.opt()])

        # ---- x2 = rs_out + x + o_b ; LN2 ; FF (local rows) ----
        with tc.tile_pool(name="ff", bufs=1) as ff, \
             tc.tile_pool(name="ffs", bufs=3) as ffsp, \
             tc.tile_pool(name="w1p", bufs=9) as w1p, \
             tc.tile_pool(name="w2p", bufs=4) as w2p, \
             tc.tile_pool(name="ps_f1", bufs=2, space="PSUM") as ps_f1, \
             tc.tile_pool(name="ps_f2", bufs=1, space="PSUM") as ps_f2:

            ln2p = tc.alloc_tile_pool(name="ln2p", bufs=1)
            x2_tiles, xn2_tiles = [], []
            for i in range(4):
                rl = ffsp.tile([128, H], F32, tag="rsld")
                nc.sync.dma_start(out=rl[:], in_=rs_out[i * 128:(i + 1) * 128, :])
                x2 = ff.tile([128, H], F32, tag=f"x2{i}")
                nc.vector.tensor_add(x2[:], rl[:], xm_tiles[i][:])
                nc.vector.tensor_add(x2[:], x2[:], ob_bc[:])
                x2_tiles.append(x2)
                ns = ffsp.tile([128, 1], F32, tag="negsum2")
                nc.vector.reduce_sum(out=ns[:], in_=x2[:], axis=AXX, negate=True)
                nm = ffsp.tile([128, 1], F32, tag="negmean2")
                nc.scalar.mul(nm[:], ns[:], 1.0 / H)
                xn2 = ln2p.tile([128, H], F32, tag=f"xn2{i}")
                nc.vector.scalar_tensor_tensor(
                    out=xn2[:], in0=x2[:], scalar=nm[:], in1=fnw_bc[:],
                    op0=ALU.add, op1=ALU.mult)
                nc.vector.tensor_add(xn2[:], xn2[:], fnb_bc[:])
                xn2_tiles.append(xn2)

            xn2T = []
            for hc in range(HC):
                xt = ff.tile([128, SM], F32R, tag=f"xn2T{hc}")
                for si in range(4):
                    ps = ps_f1.tile([128, 512], F32, tag="f1")
                    nc.tensor.transpose(ps[:, 0:128],
                                        xn2_tiles[si][:, hc * 128:(hc + 1) * 128],
                                        ident[:])
                    nc.vector.tensor_copy(xt[:, si * 128:(si + 1) * 128],
                                          ps[:, 0:128])
                xn2T.append(xt)
            ln2p.release()

            hT = [ff.tile([128, SM], F32R, tag=f"hT{i}", name=f"hT{i}") for i in range(32)]
            for ib in range(8):
                w1t = []
                for hc in range(HC):
                    t = w1p.tile([128, 512], F32R, tag="w1")
                    nc.sync.dma_start(
                        out=t[:],
                        in_=ff1wT[hc * 128:(hc + 1) * 128,
                                  ib * 512:(ib + 1) * 512].bitcast(F32R))
                    w1t.append(t)
                for sub in range(4):
                    it = ib * 4 + sub
                    ps = ps_f1.tile([128, 512], F32, tag="f1")
                    for hc in range(HC):
                        nc.tensor.matmul(ps[:],
                                         w1t[hc][:, sub * 128:(sub + 1) * 128],
                                         xn2T[hc][:], start=(hc == 0), stop=(hc == 7))
                    nc.scalar.activation(hT[it][:], ps[:], AF.Relu,
                                         bias=ffb1_pp[:, it:it + 1])

            y_sb = [ff.tile([128, H], F32, tag=f"y{i}", name=f"ysb{i}") for i in range(4)]
            for hf in range(2):
                yps = [ps_f2.tile([128, 512], F32, name=f"yps{hf}_{i}", tag=f"yps{i}", bufs=1) for i in range(4)]
                for ic in range(32):
                    w2t = w2p.tile([128, 512], F32R, tag="w2")
                    nc.sync.dma_start(
                        out=w2t[:],
                        in_=ff2wT[ic * 128:(ic + 1) * 128,
                                  hf * 512:(hf + 1) * 512].bitcast(F32R))
                    for s4 in range(4):
                        nc.tensor.matmul(yps[s4][:],
                                         hT[ic][:, s4 * 128:(s4 + 1) * 128],
                                         w2t[:], start=(ic == 0), stop=(ic == 31),
                                         skip_group_check=True)
                for s4 in range(4):
                    sl = slice(hf * 512, (hf + 1) * 512)
                    nc.vector.tensor_add(y_sb[s4][:, sl], yps[s4][:],
                                         x2_tiles[s4][:, sl])
                    nc.vector.tensor_add(y_sb[s4][:, sl], y_sb[s4][:, sl],
                                         ffb2_bc[:, sl])
            for s4 in range(4):
                nc.sync.dma_start(out=y[s4 * 128:(s4 + 1) * 128, :], in_=y_sb[s4][:])

        xmp.release()
        dram.release()
        cst.release()

    nc.compile()
    return nc


def make_in_maps(inputs):
    f = lambda a: np.ascontiguousarray(np.asarray(a, dtype=np.float32))
    x = f(inputs["x"])
    q_w, k_w, v_w = f(inputs["q_w"]), f(inputs["k_w"]), f(inputs["v_w"])
    o_w = f(inputs["o_w"])
    ff1_w, ff2_w = f(inputs["ff1_w"]), f(inputs["ff2_w"])
    ff1wT = np.ascontiguousarray(ff1_w.T)
    ff2wT = np.ascontiguousarray(ff2_w.T)
    ff1b = np.ascontiguousarray(f(inputs["ff1_b"]).reshape(32, 128))
    row = lambda a: np.ascontiguousarray(a.reshape(1, -1))
    in_maps = []
    for m in range(NC):
        dm = slice(m * DM, (m + 1) * DM)
        wqkvT = np.ascontiguousarray(
            np.concatenate([q_w[dm].T, k_w[dm].T, v_w[dm].T], axis=1))
        bqkv = np.ascontiguousarray(np.concatenate(
            [f(inputs["q_b"])[dm], f(inputs["k_b"])[dm], f(inputs["v_b"])[dm]]
        ).reshape(1, -1))
        in_maps.append({
            "x_m": np.ascontiguousarray(x[m * SM:(m + 1) * SM]),
            "wqkvT": wqkvT,
            "bqkv": bqkv,
            "owT": np.ascontiguousarray(o_w[:, dm].T),
            "ob": row(f(inputs["o_b"])),
            "anw": row(f(inputs["an_w"])), "anb": row(f(inputs["an_b"])),
            "fnw": row(f(inputs["fn_w"])), "fnb": row(f(inputs["fn_b"])),
            "ff1wT": ff1wT, "ff1b": ff1b,
            "ff2wT": ff2wT, "ffb2": row(f(inputs["ff2_b"])),
        })
    return in_maps


def kernel(**inputs) -> np.ndarray:
    from concourse.bass_utils import run_bass_kernel_spmd
    if "nc" not in _CACHE:
        _CACHE["nc"] = build_nc()
    nc = _CACHE["nc"]
    in_maps = make_in_maps(inputs)
    res = run_bass_kernel_spmd(nc, in_maps, core_ids=list(range(NC)))
    return np.concatenate([res.results[m]["y"] for m in range(NC)], axis=0)



# revision 2
# speedup vs baseline: 1.0057x; 1.0057x over previous
"""BertBlock (mean-only LN folded into weights) on 8 trn2 NeuronCores.

Design vs baseline:
- Host folds LN1 into effective QKV weights and LN2 into effective FF1
  weights (mean-only LN is linear), pre-transposes x and all weights,
  casts matmul streams to bf16. No LN compute on device at all.
- No AllGather / ReduceScatter. Each core receives full x^T (bf16) and
  computes Q/K/V for its 2 heads over the whole sequence. After
  attention, one small AllToAll (bf16 ctx + denominators, 520KB) moves
  to row-parallel layout; softmax normalization is deferred to after
  the A2A (single reciprocal on [16,512] instead of 16x [1,512]).
- Out-proj + FF run row-locally (512 rows/core) with full weights.
- EXP runs in 1536-wide chunks spanning 3 PSUM banks to amortize the
  ~352-cycle fixed overhead per ACTIVATE.
"""
import sys

sys.path.insert(0, '/opt/trn_rl_repo')

import numpy as np
import concourse.bass as bass
from concourse import bacc
import concourse.mybir as mybir
import concourse.tile as tile
from concourse.masks import make_identity

S = 4096          # sequence length
H = 1024          # hidden
I_ = 4096         # ffn inner
NH = 16           # heads
HD = 64           # head dim
NC = 8            # cores
SM = S // NC      # 512 rows per core
DM = 128          # inner dims per core (2 heads x 64)
HC = H // 128     # 8 hidden chunks
ST = S // 512     # 8 s-tiles of 512
NT = S // 128     # 32 key tiles of 128
F32 = mybir.dt.float32
F32R = mybir.dt.float32r
BF16 = mybir.dt.bfloat16
AF = mybir.ActivationFunctionType
ALU = mybir.AluOpType

_CACHE = {}


def build_nc():
    nc = bacc.Bacc(None, target_bir_lowering=False, debug=False)
    P = lambda name, shape, dt=F32: nc.declare_dram_parameter(name, shape, dt, isOutput=False)
    xT = P("xT", [H, S], BF16)            # full x^T, bf16
    xTm = P("xTm", [H, SM])               # my columns of x^T, f32 (residual)
    wqkvT = P("wqkvT", [H, 3 * DM], BF16)  # effective (LN-folded) qkv weights
    bqkv = P("bqkv", [DM, 3])             # per-partition effective biases
    owT = P("owT", [H, H], BF16)          # o_w.T (full)
    obp = P("obp", [128, HC])             # o_b per-partition layout
    ff1T = P("ff1T", [32, 128, H], BF16)  # effective ff1, ic-major blocks
    ffb1 = P("ffb1", [128, 32])
    ff2T = P("ff2T", [I_, H], BF16)
    ffb2 = P("ffb2", [128, HC])
    y = nc.declare_dram_parameter("y", [SM, H], F32, isOutput=True)

    with tile.TileContext(nc) as tc:
        cst = tc.alloc_tile_pool(name="cst", bufs=1)
        dram = tc.alloc_tile_pool(name="dram", bufs=1, space="DRAM")
        a2a_in = [dram.tile([NC, HD + 1, SM], BF16, name=f"a2a_in{h}")
                  for h in range(2)]
        a2a_out = [dram.tile([NC, HD + 1, SM], BF16, name=f"a2a_out{h}")
                   for h in range(2)]

        # ---- constants ----
        identb = cst.tile([128, 128], BF16)
        make_identity(nc, identb)
        identf = cst.tile([128, 128], F32)
        make_identity(nc, identf)
        # E1: ones row, lhsT for broadcasting a [1,512] reciprocal row to 64
        # partitions via PE
        E1 = cst.tile([1, 128], BF16)
        nc.gpsimd.memset(E1, 1.0)

        bqkv_sb = cst.tile([DM, 3], F32)
        nc.sync.dma_start(out=bqkv_sb[:], in_=bqkv[:])
        obp_sb = cst.tile([128, HC], F32)
        nc.sync.dma_start(out=obp_sb[:], in_=obp[:])
        ffb1_sb = cst.tile([128, 32], F32)
        nc.sync.dma_start(out=ffb1_sb[:], in_=ffb1[:])
        ffb2_sb = cst.tile([128, HC], F32)
        nc.sync.dma_start(out=ffb2_sb[:], in_=ffb2[:])

        # ---- persistent SBUF tensors ----
        # LIFO release order: xtp (after QKV) -> atp (after attention) ->
        # ff2p/wp (end). ff2p is created early so it sits below atp/xtp on
        # the pool stack; its DMAs are issued after xtp releases.
        wp = tc.alloc_tile_pool(name="wp", bufs=1)     # weights
        ff2p = tc.alloc_tile_pool(name="ff2p", bufs=1)
        atp = tc.alloc_tile_pool(name="atp", bufs=1)   # attention state
        xtp = tc.alloc_tile_pool(name="xtp", bufs=1)   # released after QKV

        # x^T: issue DMAs column-quarter-major so early r-tiles land first
        xT_t = [xtp.tile([128, S], BF16, tag=f"xT{hc}", name=f"xT{hc}")
                for hc in range(HC)]
        for q in range(4):
            cs = slice(q * 1024, (q + 1) * 1024)
            for hc in range(HC):
                nc.sync.dma_start(out=xT_t[hc][:, cs],
                                  in_=xT[hc * 128:(hc + 1) * 128, cs])
        wqkv_t = [wp.tile([128, 3 * DM], BF16, tag=f"wqkv{hc}", name=f"wqkv{hc}") for hc in range(HC)]
        for hc in range(HC):
            nc.sync.dma_start(out=wqkv_t[hc][:], in_=wqkvT[hc * 128:(hc + 1) * 128, :])
        xTm_t = [wp.tile([128, SM], F32, tag=f"xTm{hc}", name=f"xTm{hc}") for hc in range(HC)]
        for hc in range(HC):
            nc.sync.dma_start(out=xTm_t[hc][:], in_=xTm[hc * 128:(hc + 1) * 128, :])
        owT_t = [wp.tile([128, H], BF16, tag=f"owT{j}", name=f"owT{j}") for j in range(HC)]
        for j in range(HC):
            nc.sync.dma_start(out=owT_t[j][:], in_=owT[j * 128:(j + 1) * 128, :])
        # attention state: both heads packed [h0 rows 0:64, h1 rows 64:128];
        # scores contract over K=64 partition windows (base 0 / 64 allowed)
        QT2 = atp.tile([128, S], BF16, tag="QT2", name="QT2")
        KT2 = atp.tile([128, S], BF16, tag="KT2", name="KT2")
        # vaug[h]: [s-part 128, 32 tiles x (64 v-dims + ones col)]
        vaug = [atp.tile([128, NT * 65], BF16, tag=f"va{h}", name=f"va{h}")
                for h in range(2)]
        for h in range(2):
            for t in range(NT):
                nc.gpsimd.memset(vaug[h][:, t * 65 + HD:t * 65 + 65], 1.0)

        # ---- Phase 1: QKV projections (LN folded into weights) ----
        with tc.tile_pool(name="p1v", bufs=3) as p1v, \
             tc.tile_pool(name="ps_q", bufs=3, space="PSUM") as ps_q, \
             tc.tile_pool(name="ps_t", bufs=3, space="PSUM") as ps_t:
            for r in range(ST):
                rs = slice(r * 512, (r + 1) * 512)
                for j, dest in ((0, QT2), (1, KT2)):
                    ps = ps_q.tile([128, 512], F32, tag="qkv")
                    for hc in range(HC):
                        nc.tensor.matmul(ps[:], wqkv_t[hc][:, j * 128:(j + 1) * 128],
                                         xT_t[hc][:, rs], start=(hc == 0), stop=(hc == 7))
                    nc.vector.tensor_scalar_add(dest[:, rs], ps[:],
                                                bqkv_sb[:, j:j + 1])
                ps = ps_q.tile([128, 512], F32, tag="qkv")
                for hc in range(HC):
                    nc.tensor.matmul(ps[:], wqkv_t[hc][:, 2 * 128:3 * 128],
                                     xT_t[hc][:, rs], start=(hc == 0), stop=(hc == 7))
                vtmp = p1v.tile([128, 512], BF16, tag="vtmp")
                nc.vector.tensor_scalar_add(vtmp[:], ps[:], bqkv_sb[:, 2:3])
                for tb in range(4):
                    t = r * 4 + tb
                    pst = ps_t.tile([128, 128], BF16, tag="vt")
                    nc.tensor.transpose(pst[:], vtmp[:, tb * 128:(tb + 1) * 128],
                                        identb[:])
                    nc.vector.tensor_copy(vaug[0][:, t * 65:t * 65 + HD], pst[:, 0:HD])
                    nc.vector.tensor_copy(vaug[1][:, t * 65:t * 65 + HD],
                                          pst[:, HD:2 * HD])
        xtp.release()

        # FF2 weights: resident; loads overlap the attention phase
        ff2_t = [ff2p.tile([128, H], BF16, tag=f"w2_{ic}", name=f"w2_{ic}")
                 for ic in range(32)]
        for ic in range(32):
            nc.sync.dma_start(out=ff2_t[ic][:], in_=ff2T[ic * 128:(ic + 1) * 128, :])

        # ---- Phase 2: attention (2 chains in flight, 3-wide exp) ----
        GROUPS = [tuple(range(g, min(g + 3, NT))) for g in range(0, NT, 3)]
        # h-major: all h=0 chains finish first so their AllToAll overlaps the
        # h=1 half of attention
        chains = [(h, r) for h in range(2) for r in range(ST)]
        with tc.tile_pool(name="exb", bufs=4) as exb, \
             tc.tile_pool(name="stg", bufs=4) as stg, \
             tc.tile_pool(name="ps_s", bufs=2, space="PSUM") as ps_s, \
             tc.tile_pool(name="ps_c", bufs=1, space="PSUM") as ps_c:
            for ci in range(0, len(chains), 2):
                grp = chains[ci:ci + 2]
                cps_l = [ps_c.tile([128, 512], F32, name=f"cps{ci + i}", tag=f"cps{i}")
                         for i in range(len(grp))]
                for ts in GROUPS:
                    exl = []
                    for (h, r), cps in zip(grp, cps_l):
                        rs = slice(r * 512, (r + 1) * 512)
                        n = len(ts) * 512
                        hs = slice(h * HD, (h + 1) * HD)
                        sps = ps_s.tile([128, 1536], F32, tag="sps")
                        for i, t in enumerate(ts):
                            nc.tensor.matmul(sps[:, i * 512:(i + 1) * 512],
                                             KT2[hs, t * 128:(t + 1) * 128],
                                             QT2[hs, rs], start=True, stop=True)
                        ex = exb.tile([128, 1536], BF16, tag="ex")
                        nc.scalar.activation(ex[:, 0:n], sps[:, 0:n], AF.Exp,
                                             scale=0.125)
                        exl.append(ex)
                    for (h, r), cps, ex in zip(grp, cps_l, exl):
                        for i, t in enumerate(ts):
                            nc.tensor.matmul(cps[0:HD + 1, :],
                                             vaug[h][:, t * 65:t * 65 + 65],
                                             ex[:, i * 512:(i + 1) * 512],
                                             start=(t == 0), stop=(t == NT - 1),
                                             skip_group_check=True)
                # drain unnormalized ctx + denominator to A2A buffer
                for (h, r), cps in zip(grp, cps_l):
                    st = stg.tile([HD + 1, 512], BF16, tag="st")
                    nc.vector.tensor_copy(st[:], cps[0:HD + 1, :])
                    nc.sync.dma_start(out=a2a_in[h][r, :, :], in_=st[:, :])
                if ci * 2 + len(grp) * 2 == len(chains):
                    pass  # last pair handled below
                if ci + 2 == len(chains) // 2:
                    # all h=0 chains drained: exchange them now, overlapped
                    # with the h=1 half of attention
                    nc.gpsimd.collective_compute(
                        "AllToAll", ALU.bypass,
                        replica_groups=[list(range(NC))],
                        ins=[a2a_in[0].opt()], outs=[a2a_out[0].opt()])

        atp.release()
        nc.gpsimd.collective_compute(
            "AllToAll", ALU.bypass, replica_groups=[list(range(NC))],
            ins=[a2a_in[1].opt()], outs=[a2a_out[1].opt()])

        # ---- Phase 3: normalize + out-proj (row-local) ----
        ffp = tc.alloc_tile_pool(name="ffp", bufs=1)
        x2f = [ffp.tile([128, SM], F32, tag=f"x2f{oh}", name=f"x2f{oh}")
               for oh in range(HC)]
        x2b = [ffp.tile([128, SM], BF16, tag=f"x2b{oh}", name=f"x2b{oh}")
               for oh in range(HC)]
        with tc.tile_pool(name="p3", bufs=4) as p3, \
             tc.tile_pool(name="nmp", bufs=1) as nmp, \
             tc.tile_pool(name="ps_n", bufs=2, space="PSUM") as ps_n, \
             tc.tile_pool(name="ps_o", bufs=2, space="PSUM") as ps_o:
            nm = [nmp.tile([128, SM], BF16, tag=f"nm{j}", name=f"nm{j}") for j in range(NC)]
            for h in range(2):
                for j in range(NC):
                    cj = p3.tile([HD, SM], BF16, tag="cj")
                    nc.sync.dma_start(out=cj[:], in_=a2a_out[h][j, 0:HD, :])
                    dnbi = p3.tile([1, 512], BF16, tag="dnbi")
                    nc.sync.dma_start(out=dnbi[:], in_=a2a_out[h][j, HD:HD + 1, :])
                    # broadcast denom to 64 partitions, then approx-reciprocal
                    dps = ps_n.tile([HD, 512], F32, tag="bps")
                    nc.tensor.matmul(dps[:], E1[0:1, 0:HD], dnbi[0:1, :],
                                     start=True, stop=True)
                    rcf = p3.tile([HD, 512], F32, tag="rcf")
                    nc.vector.reciprocal_approx_fast(rcf[:], dps[:])
                    nc.vector.tensor_mul(nm[j][h * HD:(h + 1) * HD, :],
                                         cj[:], rcf[:])
            for oh in range(HC):
                po = ps_o.tile([128, 512], F32, tag="po")
                for j in range(NC):
                    nc.tensor.matmul(po[:], owT_t[j][:, oh * 128:(oh + 1) * 128],
                                     nm[j][:], start=(j == 0), stop=(j == 7))
                nc.vector.scalar_tensor_tensor(
                    out=x2f[oh][:], in0=po[:], scalar=obp_sb[:, oh:oh + 1],
                    in1=xTm_t[oh][:], op0=ALU.add, op1=ALU.add)
                nc.scalar.activation(x2b[oh][:], x2f[oh][:], AF.Copy)

        # ---- Phase 4+5: FF (row-local, LN2 folded into ff1) ----
        with tc.tile_pool(name="hp", bufs=1) as hp, \
             tc.tile_pool(name="yp", bufs=2) as ypool, \
             tc.tile_pool(name="yrowp", bufs=1) as yrowp, \
             tc.tile_pool(name="ps_f", bufs=3, space="PSUM") as ps_f, \
             tc.tile_pool(name="ps_y", bufs=3, space="PSUM") as ps_y:
            hT = [hp.tile([128, SM], BF16, tag=f"hT{ic}", name=f"hT{ic}")
                  for ic in range(32)]
            w1p = tc.alloc_tile_pool(name="w1p", bufs=6)
            for ic in range(32):
                w1 = w1p.tile([128, H], BF16, tag="w1")
                nc.sync.dma_start(out=w1[:], in_=ff1T[ic, :, :])
                ps = ps_f.tile([128, 512], F32, tag="f1")
                for hc in range(HC):
                    nc.tensor.matmul(ps[:], w1[:, hc * 128:(hc + 1) * 128],
                                     x2b[hc][:], start=(hc == 0), stop=(hc == 7))
                nc.scalar.activation(hT[ic][:], ps[:], AF.Relu,
                                     bias=ffb1_sb[:, ic:ic + 1])
            w1p.release()
            yrow = [yrowp.tile([128, H], F32, tag=f"yr{s4}", name=f"yr{s4}")
                    for s4 in range(4)]
            for oh in range(HC):
                yps = ps_f.tile([128, 512], F32, tag="f1")
                for ic in range(32):
                    nc.tensor.matmul(yps[:], ff2_t[ic][:, oh * 128:(oh + 1) * 128],
                                     hT[ic][:], start=(ic == 0), stop=(ic == 31),
                                     skip_group_check=True)
                yf = ypool.tile([128, 512], F32, tag="yf")
                nc.vector.scalar_tensor_tensor(
                    out=yf[:], in0=yps[:], scalar=ffb2_sb[:, oh:oh + 1],
                    in1=x2f[oh][:], op0=ALU.add, op1=ALU.add)
                for s4 in range(4):
                    pst = ps_y.tile([128, 128], F32, tag="yt")
                    nc.tensor.transpose(pst[:], yf[:, s4 * 128:(s4 + 1) * 128],
                                        identf[:])
                    nc.vector.tensor_copy(yrow[s4][:, oh * 128:(oh + 1) * 128],
                                          pst[:])
            for s4 in range(4):
                nc.sync.dma_start(out=y[s4 * 128:(s4 + 1) * 128, :], in_=yrow[s4][:])

        ffp.release()
        ff2p.release()
        wp.release()
        dram.release()
        cst.release()

    nc.compile()
    return nc


def make_in_maps(inputs):
    import ml_dtypes
    bf16 = ml_dtypes.bfloat16
    f = lambda a: np.asarray(a, dtype=np.float32)
    x = f(inputs["x"])
    an_w, an_b = f(inputs["an_w"]), f(inputs["an_b"])
    fn_w, fn_b = f(inputs["fn_w"]), f(inputs["fn_b"])
    q_w, k_w, v_w = f(inputs["q_w"]), f(inputs["k_w"]), f(inputs["v_w"])
    q_b, k_b, v_b = f(inputs["q_b"]), f(inputs["k_b"]), f(inputs["v_b"])
    o_w, o_b = f(inputs["o_w"]), f(inputs["o_b"])
    ff1_w, ff1_b = f(inputs["ff1_w"]), f(inputs["ff1_b"])
    ff2_w, ff2_b = f(inputs["ff2_w"]), f(inputs["ff2_b"])

    def fold(W, w, b, bias):
        # y = W @ (diag(w)(x - mean(x)) + b) + bias  ==  Aeff @ x + c
        A = W * w[None, :]
        Aeff = A - A.mean(axis=1, keepdims=True)
        c = W @ b + bias
        return Aeff.astype(np.float32), c.astype(np.float32)

    qA, qc = fold(q_w, an_w, an_b, q_b)
    kA, kc = fold(k_w, an_w, an_b, k_b)
    vA, vc = fold(v_w, an_w, an_b, v_b)
    f1A, f1c = fold(ff1_w, fn_w, fn_b, ff1_b)

    xT_bf = np.ascontiguousarray(x.T.astype(bf16))
    xT_f = np.ascontiguousarray(x.T)
    owT = np.ascontiguousarray(o_w.T.astype(bf16))
    obp = np.ascontiguousarray(o_b.reshape(HC, 128).T)
    # [32, 128, 1024]: per inter-chunk ic, one contiguous block holding the
    # 8 h-chunk lhsT tiles side by side: [ic, p, hc*128+m] = f1A.T[hc*128+p,
    # ic*128+m]
    f1T = f1A.T  # [1024 h, 4096 inter]
    ff1T = np.ascontiguousarray(
        f1T.reshape(8, 128, 32, 128).transpose(2, 1, 0, 3).reshape(32, 128, 1024)
        .astype(bf16))
    ffb1 = np.ascontiguousarray(f1c.reshape(32, 128).T)
    ff2T = np.ascontiguousarray(ff2_w.T.astype(bf16))
    ffb2 = np.ascontiguousarray(ff2_b.reshape(HC, 128).T)

    in_maps = []
    for m in range(NC):
        dm = slice(m * DM, (m + 1) * DM)
        wqkvT = np.ascontiguousarray(
            np.concatenate([qA[dm].T, kA[dm].T, vA[dm].T], axis=1).astype(bf16))
        bqkv = np.ascontiguousarray(
            np.stack([qc[dm], kc[dm], vc[dm]], axis=1))
        in_maps.append({
            "xT": xT_bf,
            "xTm": np.ascontiguousarray(xT_f[:, m * SM:(m + 1) * SM]),
            "wqkvT": wqkvT,
            "bqkv": bqkv,
            "owT": owT,
            "obp": obp,
            "ff1T": ff1T,
            "ffb1": ffb1,
            "ff2T": ff2T,
            "ffb2": ffb2,
        })
    return in_maps


def kernel(**inputs) -> np.ndarray:
    from concourse.bass_utils import run_bass_kernel_spmd
    if "nc" not in _CACHE:
        _CACHE["nc"] = build_nc()
    nc = _CACHE["nc"]
    in_maps = make_in_maps(inputs)
    res = run_bass_kernel_spmd(nc, in_maps, core_ids=list(range(NC)))
    return np.concatenate([res.results[m]["y"] for m in range(NC)], axis=0)


# revision 3
# speedup vs baseline: 1.0158x; 1.0101x over previous
"""BertBlock (mean-only LN folded into weights) on 8 trn2 NeuronCores.

Design vs baseline:
- Host folds LN1 into effective QKV weights and LN2 into effective FF1
  weights (mean-only LN is linear), pre-transposes x and all weights,
  casts matmul streams to bf16. No LN compute on device at all.
- No AllGather / ReduceScatter. Each core receives full x^T (bf16) and
  computes Q/K/V for its 2 heads over the whole sequence. After
  attention, one small AllToAll (bf16 ctx + denominators, 520KB) moves
  to row-parallel layout; softmax normalization is deferred to after
  the A2A (single reciprocal on [16,512] instead of 16x [1,512]).
- Out-proj + FF run row-locally (512 rows/core) with full weights.
- EXP runs in 1536-wide chunks spanning 3 PSUM banks to amortize the
  ~352-cycle fixed overhead per ACTIVATE.
"""
import sys

sys.path.insert(0, '/opt/trn_rl_repo')

import numpy as np
import concourse.bass as bass
from concourse import bacc
import concourse.mybir as mybir
import concourse.tile as tile
from concourse.masks import make_identity

S = 4096          # sequence length
H = 1024          # hidden
I_ = 4096         # ffn inner
NH = 16           # heads
HD = 64           # head dim
NC = 8            # cores
SM = S // NC      # 512 rows per core
DM = 128          # inner dims per core (2 heads x 64)
HC = H // 128     # 8 hidden chunks
ST = S // 512     # 8 s-tiles of 512
NT = S // 128     # 32 key tiles of 128
F32 = mybir.dt.float32
F32R = mybir.dt.float32r
BF16 = mybir.dt.bfloat16
AF = mybir.ActivationFunctionType
ALU = mybir.AluOpType

_CACHE = {}


def build_nc():
    nc = bacc.Bacc(None, target_bir_lowering=False, debug=False)
    P = lambda name, shape, dt=F32: nc.declare_dram_parameter(name, shape, dt, isOutput=False)
    xT = P("xT", [H, S], BF16)            # full x^T, bf16
    xTm = P("xTm", [H, SM])               # my columns of x^T, f32 (residual)
    wqkvT = P("wqkvT", [H, 3 * DM], BF16)  # effective (LN-folded) qkv weights
    bqkv = P("bqkv", [DM, 3])             # per-partition effective biases
    owT = P("owT", [H, H], BF16)          # o_w.T (full)
    obp = P("obp", [128, HC])             # o_b per-partition layout
    ff1T = P("ff1T", [32, 128, H], BF16)  # effective ff1, ic-major blocks
    ffb1 = P("ffb1", [128, 32])
    ff2T = P("ff2T", [I_, H], BF16)
    ffb2 = P("ffb2", [128, HC])
    y = nc.declare_dram_parameter("y", [SM, H], F32, isOutput=True)

    with tile.TileContext(nc) as tc:
        cst = tc.alloc_tile_pool(name="cst", bufs=1)
        dram = tc.alloc_tile_pool(name="dram", bufs=1, space="DRAM")
        a2a_in = [dram.tile([NC, HD + 1, SM], BF16, name=f"a2a_in{h}")
                  for h in range(2)]
        a2a_out = [dram.tile([NC, HD + 1, SM], BF16, name=f"a2a_out{h}")
                   for h in range(2)]

        # ---- constants ----
        identb = cst.tile([128, 128], BF16)
        make_identity(nc, identb)
        identf = cst.tile([128, 128], F32)
        make_identity(nc, identf)
        # E1: ones row, lhsT for broadcasting a [1,512] reciprocal row to 64
        # partitions via PE
        E1 = cst.tile([1, 128], BF16)
        nc.gpsimd.memset(E1, 1.0)

        bqkv_sb = cst.tile([DM, 3], F32)
        nc.sync.dma_start(out=bqkv_sb[:], in_=bqkv[:])
        obp_sb = cst.tile([128, HC], F32)
        nc.sync.dma_start(out=obp_sb[:], in_=obp[:])
        ffb1_sb = cst.tile([128, 32], F32)
        nc.sync.dma_start(out=ffb1_sb[:], in_=ffb1[:])
        ffb2_sb = cst.tile([128, HC], F32)
        nc.sync.dma_start(out=ffb2_sb[:], in_=ffb2[:])

        # ---- persistent SBUF tensors ----
        # LIFO release order: xtp (after QKV) -> atp (after attention) ->
        # ff2p/wp (end). ff2p is created early so it sits below atp/xtp on
        # the pool stack; its DMAs are issued after xtp releases.
        wp = tc.alloc_tile_pool(name="wp", bufs=1)     # weights
        ff2p = tc.alloc_tile_pool(name="ff2p", bufs=1)
        atp = tc.alloc_tile_pool(name="atp", bufs=1)   # attention state

        wqkv_t = [wp.tile([128, 3 * DM], BF16, tag=f"wqkv{hc}", name=f"wqkv{hc}") for hc in range(HC)]
        for hc in range(HC):
            nc.sync.dma_start(out=wqkv_t[hc][:], in_=wqkvT[hc * 128:(hc + 1) * 128, :])
        xTm_t = [wp.tile([128, SM], F32, tag=f"xTm{hc}", name=f"xTm{hc}") for hc in range(HC)]
        for hc in range(HC):
            nc.sync.dma_start(out=xTm_t[hc][:], in_=xTm[hc * 128:(hc + 1) * 128, :])
        owT_t = [wp.tile([128, H], BF16, tag=f"owT{j}", name=f"owT{j}") for j in range(HC)]
        for j in range(HC):
            nc.sync.dma_start(out=owT_t[j][:], in_=owT[j * 128:(j + 1) * 128, :])
        # attention state: both heads packed [h0 rows 0:64, h1 rows 64:128];
        # scores contract over K=64 partition windows (base 0 / 64 allowed)
        QT2 = atp.tile([128, S], BF16, tag="QT2", name="QT2")
        KT2 = atp.tile([128, S], BF16, tag="KT2", name="KT2")
        # vaug[h]: [s-part 128, 32 tiles x (64 v-dims + ones col)]
        vaug = [atp.tile([128, NT * 65], BF16, tag=f"va{h}", name=f"va{h}")
                for h in range(2)]
        for h in range(2):
            for t in range(NT):
                nc.gpsimd.memset(vaug[h][:, t * 65 + HD:t * 65 + 65], 1.0)

        # ---- Phase 1: QKV projections (LN folded into weights) ----
        with tc.tile_pool(name="p1v", bufs=3) as p1v, \
             tc.tile_pool(name="xsp", bufs=3) as xsp, \
             tc.tile_pool(name="ps_q", bufs=3, space="PSUM") as ps_q, \
             tc.tile_pool(name="ps_t", bufs=3, space="PSUM") as ps_t:
            for r in range(ST):
                rs = slice(r * 512, (r + 1) * 512)
                xr = []
                for hc in range(HC):
                    t = xsp.tile([128, 512], BF16, tag=f"x{hc}")
                    nc.sync.dma_start(out=t[:], in_=xT[hc * 128:(hc + 1) * 128, rs])
                    xr.append(t)
                for j, dest in ((0, QT2), (1, KT2)):
                    ps = ps_q.tile([128, 512], F32, tag="qkv")
                    for hc in range(HC):
                        nc.tensor.matmul(ps[:], wqkv_t[hc][:, j * 128:(j + 1) * 128],
                                         xr[hc][:], start=(hc == 0), stop=(hc == 7))
                    nc.vector.tensor_scalar_add(dest[:, rs], ps[:],
                                                bqkv_sb[:, j:j + 1])
                ps = ps_q.tile([128, 512], F32, tag="qkv")
                for hc in range(HC):
                    nc.tensor.matmul(ps[:], wqkv_t[hc][:, 2 * 128:3 * 128],
                                     xr[hc][:], start=(hc == 0), stop=(hc == 7))
                vtmp = p1v.tile([128, 512], BF16, tag="vtmp")
                nc.vector.tensor_scalar_add(vtmp[:], ps[:], bqkv_sb[:, 2:3])
                for tb in range(4):
                    t = r * 4 + tb
                    pst = ps_t.tile([128, 128], BF16, tag="vt")
                    nc.tensor.transpose(pst[:], vtmp[:, tb * 128:(tb + 1) * 128],
                                        identb[:])
                    nc.vector.tensor_copy(vaug[0][:, t * 65:t * 65 + HD], pst[:, 0:HD])
                    nc.vector.tensor_copy(vaug[1][:, t * 65:t * 65 + HD],
                                          pst[:, HD:2 * HD])
        # FF2 weights: resident; loads overlap the attention phase
        ff2_t = [ff2p.tile([128, H], BF16, tag=f"w2_{ic}", name=f"w2_{ic}")
                 for ic in range(32)]
        for ic in range(32):
            nc.sync.dma_start(out=ff2_t[ic][:], in_=ff2T[ic * 128:(ic + 1) * 128, :])

        # ---- Phase 2: attention (2 chains in flight, 3-wide exp) ----
        GROUPS = [tuple(range(g, min(g + 3, NT))) for g in range(0, NT, 3)]
        # h-major: all h=0 chains finish first so their AllToAll overlaps the
        # h=1 half of attention
        chains = [(h, r) for h in range(2) for r in range(ST)]
        with tc.tile_pool(name="exb", bufs=4) as exb, \
             tc.tile_pool(name="stg", bufs=4) as stg, \
             tc.tile_pool(name="ps_s", bufs=2, space="PSUM") as ps_s, \
             tc.tile_pool(name="ps_c", bufs=1, space="PSUM") as ps_c:
            for ci in range(0, len(chains), 2):
                grp = chains[ci:ci + 2]
                cps_l = [ps_c.tile([128, 512], F32, name=f"cps{ci + i}", tag=f"cps{i}")
                         for i in range(len(grp))]
                for ts in GROUPS:
                    exl = []
                    for (h, r), cps in zip(grp, cps_l):
                        rs = slice(r * 512, (r + 1) * 512)
                        n = len(ts) * 512
                        hs = slice(h * HD, (h + 1) * HD)
                        sps = ps_s.tile([128, 1536], F32, tag="sps")
                        for i, t in enumerate(ts):
                            nc.tensor.matmul(sps[:, i * 512:(i + 1) * 512],
                                             KT2[hs, t * 128:(t + 1) * 128],
                                             QT2[hs, rs], start=True, stop=True)
                        ex = exb.tile([128, 1536], BF16, tag="ex")
                        nc.scalar.activation(ex[:, 0:n], sps[:, 0:n], AF.Exp,
                                             scale=0.125)
                        exl.append(ex)
                    for (h, r), cps, ex in zip(grp, cps_l, exl):
                        for i, t in enumerate(ts):
                            nc.tensor.matmul(cps[0:HD + 1, :],
                                             vaug[h][:, t * 65:t * 65 + 65],
                                             ex[:, i * 512:(i + 1) * 512],
                                             start=(t == 0), stop=(t == NT - 1),
                                             skip_group_check=True)
                # drain unnormalized ctx + denominator to A2A buffer
                for (h, r), cps in zip(grp, cps_l):
                    st = stg.tile([HD + 1, 512], BF16, tag="st")
                    nc.vector.tensor_copy(st[:], cps[0:HD + 1, :])
                    nc.sync.dma_start(out=a2a_in[h][r, :, :], in_=st[:, :])
                if ci * 2 + len(grp) * 2 == len(chains):
                    pass  # last pair handled below
                if ci + 2 == len(chains) // 2:
                    # all h=0 chains drained: exchange them now, overlapped
                    # with the h=1 half of attention
                    nc.gpsimd.collective_compute(
                        "AllToAll", ALU.bypass,
                        replica_groups=[list(range(NC))],
                        ins=[a2a_in[0].opt()], outs=[a2a_out[0].opt()])

        atp.release()
        nc.gpsimd.collective_compute(
            "AllToAll", ALU.bypass, replica_groups=[list(range(NC))],
            ins=[a2a_in[1].opt()], outs=[a2a_out[1].opt()])

        # ---- Phase 3: normalize + out-proj (row-local) ----
        ffp = tc.alloc_tile_pool(name="ffp", bufs=1)
        x2f = [ffp.tile([128, SM], F32, tag=f"x2f{oh}", name=f"x2f{oh}")
               for oh in range(HC)]
        x2b = [ffp.tile([128, SM], BF16, tag=f"x2b{oh}", name=f"x2b{oh}")
               for oh in range(HC)]
        with tc.tile_pool(name="p3", bufs=4) as p3, \
             tc.tile_pool(name="nmp", bufs=1) as nmp, \
             tc.tile_pool(name="ps_n", bufs=2, space="PSUM") as ps_n, \
             tc.tile_pool(name="ps_o", bufs=2, space="PSUM") as ps_o:
            nm = [nmp.tile([128, SM], BF16, tag=f"nm{j}", name=f"nm{j}") for j in range(NC)]
            for h in range(2):
                for j in range(NC):
                    cj = p3.tile([HD, SM], BF16, tag="cj")
                    nc.sync.dma_start(out=cj[:], in_=a2a_out[h][j, 0:HD, :])
                    dnbi = p3.tile([1, 512], BF16, tag="dnbi")
                    nc.sync.dma_start(out=dnbi[:], in_=a2a_out[h][j, HD:HD + 1, :])
                    # broadcast denom to 64 partitions, then approx-reciprocal
                    dps = ps_n.tile([HD, 512], F32, tag="bps")
                    nc.tensor.matmul(dps[:], E1[0:1, 0:HD], dnbi[0:1, :],
                                     start=True, stop=True)
                    rcf = p3.tile([HD, 512], F32, tag="rcf")
                    nc.vector.reciprocal_approx_fast(rcf[:], dps[:])
                    nc.vector.tensor_mul(nm[j][h * HD:(h + 1) * HD, :],
                                         cj[:], rcf[:])
            for oh in range(HC):
                po = ps_o.tile([128, 512], F32, tag="po")
                for j in range(NC):
                    nc.tensor.matmul(po[:], owT_t[j][:, oh * 128:(oh + 1) * 128],
                                     nm[j][:], start=(j == 0), stop=(j == 7))
                nc.vector.scalar_tensor_tensor(
                    out=x2f[oh][:], in0=po[:], scalar=obp_sb[:, oh:oh + 1],
                    in1=xTm_t[oh][:], op0=ALU.add, op1=ALU.add)
                nc.scalar.activation(x2b[oh][:], x2f[oh][:], AF.Copy)

        # ---- Phase 4+5: FF (row-local, LN2 folded into ff1) ----
        with tc.tile_pool(name="hp", bufs=1) as hp, \
             tc.tile_pool(name="yp", bufs=2) as ypool, \
             tc.tile_pool(name="yrowp", bufs=1) as yrowp, \
             tc.tile_pool(name="ps_f", bufs=3, space="PSUM") as ps_f, \
             tc.tile_pool(name="ps_y", bufs=3, space="PSUM") as ps_y:
            hT = [hp.tile([128, SM], BF16, tag=f"hT{ic}", name=f"hT{ic}")
                  for ic in range(32)]
            w1p = tc.alloc_tile_pool(name="w1p", bufs=6)
            for ic in range(32):
                w1 = w1p.tile([128, H], BF16, tag="w1")
                nc.sync.dma_start(out=w1[:], in_=ff1T[ic, :, :])
                ps = ps_f.tile([128, 512], F32, tag="f1")
                for hc in range(HC):
                    nc.tensor.matmul(ps[:], w1[:, hc * 128:(hc + 1) * 128],
                                     x2b[hc][:], start=(hc == 0), stop=(hc == 7))
                nc.scalar.activation(hT[ic][:], ps[:], AF.Relu,
                                     bias=ffb1_sb[:, ic:ic + 1])
            w1p.release()
            yrow = [yrowp.tile([128, H], F32, tag=f"yr{s4}", name=f"yr{s4}")
                    for s4 in range(4)]
            for oh in range(HC):
                yps = ps_f.tile([128, 512], F32, tag="f1")
                for ic in range(32):
                    nc.tensor.matmul(yps[:], ff2_t[ic][:, oh * 128:(oh + 1) * 128],
                                     hT[ic][:], start=(ic == 0), stop=(ic == 31),
                                     skip_group_check=True)
                yf = ypool.tile([128, 512], F32, tag="yf")
                nc.vector.scalar_tensor_tensor(
                    out=yf[:], in0=yps[:], scalar=ffb2_sb[:, oh:oh + 1],
                    in1=x2f[oh][:], op0=ALU.add, op1=ALU.add)
                for s4 in range(4):
                    pst = ps_y.tile([128, 128], F32, tag="yt")
                    nc.tensor.transpose(pst[:], yf[:, s4 * 128:(s4 + 1) * 128],
                                        identf[:])
                    nc.vector.tensor_copy(yrow[s4][:, oh * 128:(oh + 1) * 128],
                                          pst[:])
            for s4 in range(4):
                nc.sync.dma_start(out=y[s4 * 128:(s4 + 1) * 128, :], in_=yrow[s4][:])

        ffp.release()
        ff2p.release()
        wp.release()
        dram.release()
        cst.release()

    nc.compile()
    return nc


def make_in_maps(inputs):
    import ml_dtypes
    bf16 = ml_dtypes.bfloat16
    f = lambda a: np.asarray(a, dtype=np.float32)
    x = f(inputs["x"])
    an_w, an_b = f(inputs["an_w"]), f(inputs["an_b"])
    fn_w, fn_b = f(inputs["fn_w"]), f(inputs["fn_b"])
    q_w, k_w, v_w = f(inputs["q_w"]), f(inputs["k_w"]), f(inputs["v_w"])
    q_b, k_b, v_b = f(inputs["q_b"]), f(inputs["k_b"]), f(inputs["v_b"])
    o_w, o_b = f(inputs["o_w"]), f(inputs["o_b"])
    ff1_w, ff1_b = f(inputs["ff1_w"]), f(inputs["ff1_b"])
    ff2_w, ff2_b = f(inputs["ff2_w"]), f(inputs["ff2_b"])

    def fold(W, w, b, bias):
        # y = W @ (diag(w)(x - mean(x)) + b) + bias  ==  Aeff @ x + c
        A = W * w[None, :]
        Aeff = A - A.mean(axis=1, keepdims=True)
        c = W @ b + bias
        return Aeff.astype(np.float32), c.astype(np.float32)

    qA, qc = fold(q_w, an_w, an_b, q_b)
    kA, kc = fold(k_w, an_w, an_b, k_b)
    vA, vc = fold(v_w, an_w, an_b, v_b)
    f1A, f1c = fold(ff1_w, fn_w, fn_b, ff1_b)

    xT_bf = np.ascontiguousarray(x.T.astype(bf16))
    xT_f = np.ascontiguousarray(x.T)
    owT = np.ascontiguousarray(o_w.T.astype(bf16))
    obp = np.ascontiguousarray(o_b.reshape(HC, 128).T)
    # [32, 128, 1024]: per inter-chunk ic, one contiguous block holding the
    # 8 h-chunk lhsT tiles side by side: [ic, p, hc*128+m] = f1A.T[hc*128+p,
    # ic*128+m]
    f1T = f1A.T  # [1024 h, 4096 inter]
    ff1T = np.ascontiguousarray(
        f1T.reshape(8, 128, 32, 128).transpose(2, 1, 0, 3).reshape(32, 128, 1024)
        .astype(bf16))
    ffb1 = np.ascontiguousarray(f1c.reshape(32, 128).T)
    ff2T = np.ascontiguousarray(ff2_w.T.astype(bf16))
    ffb2 = np.ascontiguousarray(ff2_b.reshape(HC, 128).T)

    in_maps = []
    for m in range(NC):
        dm = slice(m * DM, (m + 1) * DM)
        wqkvT = np.ascontiguousarray(
            np.concatenate([qA[dm].T, kA[dm].T, vA[dm].T], axis=1).astype(bf16))
        bqkv = np.ascontiguousarray(
            np.stack([qc[dm], kc[dm], vc[dm]], axis=1))
        in_maps.append({
            "xT": xT_bf,
            "xTm": np.ascontiguousarray(xT_f[:, m * SM:(m + 1) * SM]),
            "wqkvT": wqkvT,
            "bqkv": bqkv,
            "owT": owT,
            "obp": obp,
            "ff1T": ff1T,
            "ffb1": ffb1,
            "ff2T": ff2T,
            "ffb2": ffb2,
        })
    return in_maps


def kernel(**inputs) -> np.ndarray:
    from concourse.bass_utils import run_bass_kernel_spmd
    if "nc" not in _CACHE:
        _CACHE["nc"] = build_nc()
    nc = _CACHE["nc"]
    in_maps = make_in_maps(inputs)
    res = run_bass_kernel_spmd(nc, in_maps, core_ids=list(range(NC)))
    return np.concatenate([res.results[m]["y"] for m in range(NC)], axis=0)


# revision 4
# speedup vs baseline: 1.0288x; 1.0128x over previous
"""BertBlock (mean-only LN folded into weights) on 8 trn2 NeuronCores.

Design vs baseline:
- Host folds LN1 into effective QKV weights and LN2 into effective FF1
  weights (mean-only LN is linear), pre-transposes x and all weights,
  casts matmul streams to bf16. No LN compute on device at all.
- No AllGather / ReduceScatter. Each core receives full x^T (bf16) and
  computes Q/K/V for its 2 heads over the whole sequence. After
  attention, one small AllToAll (bf16 ctx + denominators, 520KB) moves
  to row-parallel layout; softmax normalization is deferred to after
  the A2A (single reciprocal on [16,512] instead of 16x [1,512]).
- Out-proj + FF run row-locally (512 rows/core) with full weights.
- EXP runs in 1536-wide chunks spanning 3 PSUM banks to amortize the
  ~352-cycle fixed overhead per ACTIVATE.
"""
import sys

sys.path.insert(0, '/opt/trn_rl_repo')

import numpy as np
import concourse.bass as bass
from concourse import bacc
import concourse.mybir as mybir
import concourse.tile as tile
from concourse.masks import make_identity

S = 4096          # sequence length
H = 1024          # hidden
I_ = 4096         # ffn inner
NH = 16           # heads
HD = 64           # head dim
NC = 8            # cores
SM = S // NC      # 512 rows per core
DM = 128          # inner dims per core (2 heads x 64)
HC = H // 128     # 8 hidden chunks
ST = S // 512     # 8 s-tiles of 512
NT = S // 128     # 32 key tiles of 128
F32 = mybir.dt.float32
F32R = mybir.dt.float32r
BF16 = mybir.dt.bfloat16
AF = mybir.ActivationFunctionType
ALU = mybir.AluOpType

_CACHE = {}


def build_nc():
    nc = bacc.Bacc(None, target_bir_lowering=False, debug=False)
    P = lambda name, shape, dt=F32: nc.declare_dram_parameter(name, shape, dt, isOutput=False)
    xT = P("xT", [H, S], BF16)            # full x^T, bf16
    xTm = P("xTm", [H, SM])               # my columns of x^T, f32 (residual)
    wqkvT = P("wqkvT", [H, 3 * DM], BF16)  # effective (LN-folded) qkv weights
    bqkv = P("bqkv", [DM, 3])             # per-partition effective biases
    owT = P("owT", [H, H], BF16)          # o_w.T (full)
    obp = P("obp", [128, HC])             # o_b per-partition layout
    ff1T = P("ff1T", [32, 128, H], BF16)  # effective ff1, ic-major blocks
    ffb1 = P("ffb1", [128, 32])
    ff2T = P("ff2T", [I_, H], BF16)
    ffb2 = P("ffb2", [128, HC])
    y = nc.declare_dram_parameter("y", [SM, H], F32, isOutput=True)

    with tile.TileContext(nc) as tc:
        cst = tc.alloc_tile_pool(name="cst", bufs=1)
        dram = tc.alloc_tile_pool(name="dram", bufs=1, space="DRAM")
        a2a_in = [dram.tile([NC, HD + 1, SM], BF16, name=f"a2a_in{h}")
                  for h in range(2)]
        a2a_out = [dram.tile([NC, HD + 1, SM], BF16, name=f"a2a_out{h}")
                   for h in range(2)]

        # ---- constants ----
        identb = cst.tile([128, 128], BF16)
        make_identity(nc, identb)
        identf = cst.tile([128, 128], F32)
        make_identity(nc, identf)
        # E1: ones row, lhsT for broadcasting a [1,512] reciprocal row to 64
        # partitions via PE
        E1 = cst.tile([1, 128], BF16)
        nc.gpsimd.memset(E1, 1.0)

        bqkv_sb = cst.tile([DM, 3], F32)
        nc.sync.dma_start(out=bqkv_sb[:], in_=bqkv[:])
        obp_sb = cst.tile([128, HC], F32)
        nc.sync.dma_start(out=obp_sb[:], in_=obp[:])
        ffb1_sb = cst.tile([128, 32], F32)
        nc.sync.dma_start(out=ffb1_sb[:], in_=ffb1[:])
        ffb2_sb = cst.tile([128, HC], F32)
        nc.sync.dma_start(out=ffb2_sb[:], in_=ffb2[:])

        # ---- persistent SBUF tensors ----
        # LIFO release order: xtp (after QKV) -> atp (after attention) ->
        # ff2p/wp (end). ff2p is created early so it sits below atp/xtp on
        # the pool stack; its DMAs are issued after xtp releases.
        wp = tc.alloc_tile_pool(name="wp", bufs=1)     # weights
        ff2p = tc.alloc_tile_pool(name="ff2p", bufs=1)
        atp = tc.alloc_tile_pool(name="atp", bufs=1)   # attention state

        wqkv_t = [wp.tile([128, 3 * DM], BF16, tag=f"wqkv{hc}", name=f"wqkv{hc}") for hc in range(HC)]
        for hc in range(HC):
            nc.sync.dma_start(out=wqkv_t[hc][:], in_=wqkvT[hc * 128:(hc + 1) * 128, :])
        # attention state: both heads packed [h0 rows 0:64, h1 rows 64:128];
        # scores contract over K=64 partition windows (base 0 / 64 allowed)
        QT2 = atp.tile([128, S], BF16, tag="QT2", name="QT2")
        KT2 = atp.tile([128, S], BF16, tag="KT2", name="KT2")
        # vaug[h]: [s-part 128, 32 tiles x (64 v-dims + ones col)]
        vaug = [atp.tile([128, NT * 65], BF16, tag=f"va{h}", name=f"va{h}")
                for h in range(2)]
        for h in range(2):
            for t in range(NT):
                nc.gpsimd.memset(vaug[h][:, t * 65 + HD:t * 65 + 65], 1.0)

        # ---- Phase 1: QKV projections (LN folded into weights) ----
        with tc.tile_pool(name="p1v", bufs=3) as p1v, \
             tc.tile_pool(name="xsp", bufs=3) as xsp, \
             tc.tile_pool(name="ps_q", bufs=3, space="PSUM") as ps_q, \
             tc.tile_pool(name="ps_t", bufs=3, space="PSUM") as ps_t:
            for r in range(ST):
                rs = slice(r * 512, (r + 1) * 512)
                xr = []
                for hc in range(HC):
                    t = xsp.tile([128, 512], BF16, tag=f"x{hc}")
                    nc.sync.dma_start(out=t[:], in_=xT[hc * 128:(hc + 1) * 128, rs])
                    xr.append(t)
                for j, dest in ((0, QT2), (1, KT2)):
                    ps = ps_q.tile([128, 512], F32, tag="qkv")
                    for hc in range(HC):
                        nc.tensor.matmul(ps[:], wqkv_t[hc][:, j * 128:(j + 1) * 128],
                                         xr[hc][:], start=(hc == 0), stop=(hc == 7))
                    nc.vector.tensor_scalar_add(dest[:, rs], ps[:],
                                                bqkv_sb[:, j:j + 1])
                ps = ps_q.tile([128, 512], F32, tag="qkv")
                for hc in range(HC):
                    nc.tensor.matmul(ps[:], wqkv_t[hc][:, 2 * 128:3 * 128],
                                     xr[hc][:], start=(hc == 0), stop=(hc == 7))
                vtmp = p1v.tile([128, 512], BF16, tag="vtmp")
                nc.vector.tensor_scalar_add(vtmp[:], ps[:], bqkv_sb[:, 2:3])
                for tb in range(4):
                    t = r * 4 + tb
                    pst = ps_t.tile([128, 128], BF16, tag="vt")
                    nc.tensor.transpose(pst[:], vtmp[:, tb * 128:(tb + 1) * 128],
                                        identb[:])
                    nc.vector.tensor_copy(vaug[0][:, t * 65:t * 65 + HD], pst[:, 0:HD])
                    nc.vector.tensor_copy(vaug[1][:, t * 65:t * 65 + HD],
                                          pst[:, HD:2 * HD])
        # late-phase weights: emitted after the QKV x-stream DMAs so they
        # don't delay the first matmuls; they load during attention
        xTm_t = [wp.tile([128, SM], F32, tag=f"xTm{hc}", name=f"xTm{hc}") for hc in range(HC)]
        for hc in range(HC):
            nc.sync.dma_start(out=xTm_t[hc][:], in_=xTm[hc * 128:(hc + 1) * 128, :])
        owT_t = [wp.tile([128, H], BF16, tag=f"owT{j}", name=f"owT{j}") for j in range(HC)]
        for j in range(HC):
            nc.sync.dma_start(out=owT_t[j][:], in_=owT[j * 128:(j + 1) * 128, :])
        ff2_t = [ff2p.tile([128, H], BF16, tag=f"w2_{ic}", name=f"w2_{ic}")
                 for ic in range(32)]
        for ic in range(32):
            nc.sync.dma_start(out=ff2_t[ic][:], in_=ff2T[ic * 128:(ic + 1) * 128, :])

        # ---- Phase 2: attention (2 chains in flight, 3-wide exp) ----
        GROUPS = [tuple(range(g, min(g + 3, NT))) for g in range(0, NT, 3)]
        # h-major: all h=0 chains finish first so their AllToAll overlaps the
        # h=1 half of attention
        chains = [(h, r) for h in range(2) for r in range(ST)]
        with tc.tile_pool(name="exb", bufs=4) as exb, \
             tc.tile_pool(name="stg", bufs=4) as stg, \
             tc.tile_pool(name="ps_s", bufs=2, space="PSUM") as ps_s, \
             tc.tile_pool(name="ps_c", bufs=1, space="PSUM") as ps_c:
            for ci in range(0, len(chains), 2):
                grp = chains[ci:ci + 2]
                cps_l = [ps_c.tile([128, 512], F32, name=f"cps{ci + i}", tag=f"cps{i}")
                         for i in range(len(grp))]
                for ts in GROUPS:
                    exl = []
                    for (h, r), cps in zip(grp, cps_l):
                        rs = slice(r * 512, (r + 1) * 512)
                        n = len(ts) * 512
                        hs = slice(h * HD, (h + 1) * HD)
                        sps = ps_s.tile([128, 1536], F32, tag="sps")
                        for i, t in enumerate(ts):
                            nc.tensor.matmul(sps[:, i * 512:(i + 1) * 512],
                                             KT2[hs, t * 128:(t + 1) * 128],
                                             QT2[hs, rs], start=True, stop=True)
                        ex = exb.tile([128, 1536], BF16, tag="ex")
                        nc.scalar.activation(ex[:, 0:n], sps[:, 0:n], AF.Exp,
                                             scale=0.125)
                        exl.append(ex)
                    for (h, r), cps, ex in zip(grp, cps_l, exl):
                        for i, t in enumerate(ts):
                            nc.tensor.matmul(cps[0:HD + 1, :],
                                             vaug[h][:, t * 65:t * 65 + 65],
                                             ex[:, i * 512:(i + 1) * 512],
                                             start=(t == 0), stop=(t == NT - 1),
                                             skip_group_check=True)
                # drain unnormalized ctx + denominator to A2A buffer
                for (h, r), cps in zip(grp, cps_l):
                    st = stg.tile([HD + 1, 512], BF16, tag="st")
                    nc.vector.tensor_copy(st[:], cps[0:HD + 1, :])
                    nc.sync.dma_start(out=a2a_in[h][r, :, :], in_=st[:, :])
                if ci * 2 + len(grp) * 2 == len(chains):
                    pass  # last pair handled below
                if ci + 2 == len(chains) // 2:
                    # all h=0 chains drained: exchange them now, overlapped
                    # with the h=1 half of attention
                    nc.gpsimd.collective_compute(
                        "AllToAll", ALU.bypass,
                        replica_groups=[list(range(NC))],
                        ins=[a2a_in[0].opt()], outs=[a2a_out[0].opt()])

        atp.release()
        nc.gpsimd.collective_compute(
            "AllToAll", ALU.bypass, replica_groups=[list(range(NC))],
            ins=[a2a_in[1].opt()], outs=[a2a_out[1].opt()])

        # ---- Phase 3: normalize + out-proj (row-local) ----
        ffp = tc.alloc_tile_pool(name="ffp", bufs=1)
        x2f = [ffp.tile([128, SM], F32, tag=f"x2f{oh}", name=f"x2f{oh}")
               for oh in range(HC)]
        x2b = [ffp.tile([128, SM], BF16, tag=f"x2b{oh}", name=f"x2b{oh}")
               for oh in range(HC)]
        with tc.tile_pool(name="p3", bufs=4) as p3, \
             tc.tile_pool(name="nmp", bufs=1) as nmp, \
             tc.tile_pool(name="ps_n", bufs=2, space="PSUM") as ps_n, \
             tc.tile_pool(name="ps_o", bufs=3, space="PSUM") as ps_o:
            nm = [nmp.tile([128, SM], BF16, tag=f"nm{j}", name=f"nm{j}")
                  for j in range(NC)]

            def normalize_half(h):
                for j in range(NC):
                    cj = p3.tile([HD, SM], BF16, tag="cj")
                    nc.sync.dma_start(out=cj[:], in_=a2a_out[h][j, 0:HD, :])
                    dnbi = p3.tile([1, 512], BF16, tag="dnbi")
                    nc.sync.dma_start(out=dnbi[:], in_=a2a_out[h][j, HD:HD + 1, :])
                    # broadcast denom to 64 partitions, then approx-reciprocal
                    dps = ps_n.tile([HD, 512], F32, tag="bps")
                    nc.tensor.matmul(dps[:], E1[0:1, 0:HD], dnbi[0:1, :],
                                     start=True, stop=True)
                    rcf = p3.tile([HD, 512], F32, tag="rcf")
                    nc.vector.reciprocal_approx_fast(rcf[:], dps[:])
                    nc.vector.tensor_mul(nm[j][h * HD:(h + 1) * HD, :],
                                         cj[:], rcf[:])

            # h0 half is exchanged mid-attention; its normalize + out-proj
            # partials run while the h1 AllToAll is still in flight. Each
            # partial is a closed accumulation drained to SBUF, added back in
            # during the h1 pass.
            normalize_half(0)
            x2p0 = [ffp.tile([128, SM], F32, tag=f"x2p{oh}", name=f"x2p{oh}")
                    for oh in range(HC)]
            for oh in range(HC):
                po = ps_o.tile([128, 512], F32, tag="po")
                for j in range(NC):
                    nc.tensor.matmul(po[:], owT_t[j][0:HD, oh * 128:(oh + 1) * 128],
                                     nm[j][0:HD, :], start=(j == 0), stop=(j == 7))
                nc.vector.scalar_tensor_tensor(
                    out=x2p0[oh][:], in0=po[:], scalar=obp_sb[:, oh:oh + 1],
                    in1=xTm_t[oh][:], op0=ALU.add, op1=ALU.add)
            normalize_half(1)
            for oh in range(HC):
                po = ps_o.tile([128, 512], F32, tag="po")
                for j in range(NC):
                    nc.tensor.matmul(po[:], owT_t[j][HD:128, oh * 128:(oh + 1) * 128],
                                     nm[j][HD:128, :], start=(j == 0), stop=(j == 7))
                nc.vector.tensor_add(x2f[oh][:], po[:], x2p0[oh][:])
                nc.scalar.activation(x2b[oh][:], x2f[oh][:], AF.Copy)

        # ---- Phase 4+5: FF (row-local, LN2 folded into ff1) ----
        with tc.tile_pool(name="hp", bufs=1) as hp, \
             tc.tile_pool(name="yp", bufs=2) as ypool, \
             tc.tile_pool(name="yrowp", bufs=1) as yrowp, \
             tc.tile_pool(name="ps_f", bufs=3, space="PSUM") as ps_f, \
             tc.tile_pool(name="ps_y", bufs=3, space="PSUM") as ps_y:
            hT = [hp.tile([128, SM], BF16, tag=f"hT{ic}", name=f"hT{ic}")
                  for ic in range(32)]
            w1p = tc.alloc_tile_pool(name="w1p", bufs=6)
            for ic in range(32):
                w1 = w1p.tile([128, H], BF16, tag="w1")
                nc.sync.dma_start(out=w1[:], in_=ff1T[ic, :, :])
                ps = ps_f.tile([128, 512], F32, tag="f1")
                for hc in range(HC):
                    nc.tensor.matmul(ps[:], w1[:, hc * 128:(hc + 1) * 128],
                                     x2b[hc][:], start=(hc == 0), stop=(hc == 7))
                nc.scalar.activation(hT[ic][:], ps[:], AF.Relu,
                                     bias=ffb1_sb[:, ic:ic + 1])
            w1p.release()
            yrow = [yrowp.tile([128, H], F32, tag=f"yr{s4}", name=f"yr{s4}")
                    for s4 in range(4)]
            for oh in range(HC):
                yps = ps_f.tile([128, 512], F32, tag="f1")
                for ic in range(32):
                    nc.tensor.matmul(yps[:], ff2_t[ic][:, oh * 128:(oh + 1) * 128],
                                     hT[ic][:], start=(ic == 0), stop=(ic == 31),
                                     skip_group_check=True)
                yf = ypool.tile([128, 512], F32, tag="yf")
                nc.vector.scalar_tensor_tensor(
                    out=yf[:], in0=yps[:], scalar=ffb2_sb[:, oh:oh + 1],
                    in1=x2f[oh][:], op0=ALU.add, op1=ALU.add)
                for s4 in range(4):
                    pst = ps_y.tile([128, 128], F32, tag="yt")
                    nc.tensor.transpose(pst[:], yf[:, s4 * 128:(s4 + 1) * 128],
                                        identf[:])
                    nc.vector.tensor_copy(yrow[s4][:, oh * 128:(oh + 1) * 128],
                                          pst[:])
            for s4 in range(4):
                nc.sync.dma_start(out=y[s4 * 128:(s4 + 1) * 128, :], in_=yrow[s4][:])

        ffp.release()
        ff2p.release()
        wp.release()
        dram.release()
        cst.release()

    nc.compile()
    return nc


def make_in_maps(inputs):
    import ml_dtypes
    bf16 = ml_dtypes.bfloat16
    f = lambda a: np.asarray(a, dtype=np.float32)
    x = f(inputs["x"])
    an_w, an_b = f(inputs["an_w"]), f(inputs["an_b"])
    fn_w, fn_b = f(inputs["fn_w"]), f(inputs["fn_b"])
    q_w, k_w, v_w = f(inputs["q_w"]), f(inputs["k_w"]), f(inputs["v_w"])
    q_b, k_b, v_b = f(inputs["q_b"]), f(inputs["k_b"]), f(inputs["v_b"])
    o_w, o_b = f(inputs["o_w"]), f(inputs["o_b"])
    ff1_w, ff1_b = f(inputs["ff1_w"]), f(inputs["ff1_b"])
    ff2_w, ff2_b = f(inputs["ff2_w"]), f(inputs["ff2_b"])

    def fold(W, w, b, bias):
        # y = W @ (diag(w)(x - mean(x)) + b) + bias  ==  Aeff @ x + c
        A = W * w[None, :]
        Aeff = A - A.mean(axis=1, keepdims=True)
        c = W @ b + bias
        return Aeff.astype(np.float32), c.astype(np.float32)

    qA, qc = fold(q_w, an_w, an_b, q_b)
    kA, kc = fold(k_w, an_w, an_b, k_b)
    vA, vc = fold(v_w, an_w, an_b, v_b)
    f1A, f1c = fold(ff1_w, fn_w, fn_b, ff1_b)

    xT_bf = np.ascontiguousarray(x.T.astype(bf16))
    xT_f = np.ascontiguousarray(x.T)
    owT = np.ascontiguousarray(o_w.T.astype(bf16))
    obp = np.ascontiguousarray(o_b.reshape(HC, 128).T)
    # [32, 128, 1024]: per inter-chunk ic, one contiguous block holding the
    # 8 h-chunk lhsT tiles side by side: [ic, p, hc*128+m] = f1A.T[hc*128+p,
    # ic*128+m]
    f1T = f1A.T  # [1024 h, 4096 inter]
    ff1T = np.ascontiguousarray(
        f1T.reshape(8, 128, 32, 128).transpose(2, 1, 0, 3).reshape(32, 128, 1024)
        .astype(bf16))
    ffb1 = np.ascontiguousarray(f1c.reshape(32, 128).T)
    ff2T = np.ascontiguousarray(ff2_w.T.astype(bf16))
    ffb2 = np.ascontiguousarray(ff2_b.reshape(HC, 128).T)

    in_maps = []
    for m in range(NC):
        dm = slice(m * DM, (m + 1) * DM)
        wqkvT = np.ascontiguousarray(
            np.concatenate([qA[dm].T, kA[dm].T, vA[dm].T], axis=1).astype(bf16))
        bqkv = np.ascontiguousarray(
            np.stack([qc[dm], kc[dm], vc[dm]], axis=1))
        in_maps.append({
            "xT": xT_bf,
            "xTm": np.ascontiguousarray(xT_f[:, m * SM:(m + 1) * SM]),
            "wqkvT": wqkvT,
            "bqkv": bqkv,
            "owT": owT,
            "obp": obp,
            "ff1T": ff1T,
            "ffb1": ffb1,
            "ff2T": ff2T,
            "ffb2": ffb2,
        })
    return in_maps


def kernel(**inputs) -> np.ndarray:
    from concourse.bass_utils import run_bass_kernel_spmd
    if "nc" not in _CACHE:
        _CACHE["nc"] = build_nc()
    nc = _CACHE["nc"]
    in_maps = make_in_maps(inputs)
    res = run_bass_kernel_spmd(nc, in_maps, core_ids=list(range(NC)))
    return np.concatenate([res.results[m]["y"] for m in range(NC)], axis=0)


# revision 5
# speedup vs baseline: 1.0605x; 1.0308x over previous
"""BertBlock (mean-only LN folded into weights) on 8 trn2 NeuronCores.

Design vs baseline:
- Host folds LN1 into effective QKV weights and LN2 into effective FF1
  weights (mean-only LN is linear), pre-transposes x and all weights,
  casts matmul streams to bf16. No LN compute on device at all.
- No AllGather / ReduceScatter. Each core receives full x^T (bf16) and
  computes Q/K/V for its 2 heads over the whole sequence. After
  attention, one small AllToAll (bf16 ctx + denominators, 520KB) moves
  to row-parallel layout; softmax normalization is deferred to after
  the A2A (single reciprocal on [16,512] instead of 16x [1,512]).
- Out-proj + FF run row-locally (512 rows/core) with full weights.
- EXP runs in 1536-wide chunks spanning 3 PSUM banks to amortize the
  ~352-cycle fixed overhead per ACTIVATE.
"""
import sys

sys.path.insert(0, '/opt/trn_rl_repo')

import numpy as np
import concourse.bass as bass
from concourse import bacc
import concourse.mybir as mybir
import concourse.tile as tile
from concourse.masks import make_identity

S = 4096          # sequence length
H = 1024          # hidden
I_ = 4096         # ffn inner
NH = 16           # heads
HD = 64           # head dim
NC = 8            # cores
SM = S // NC      # 512 rows per core
DM = 128          # inner dims per core (2 heads x 64)
HC = H // 128     # 8 hidden chunks
ST = S // 512     # 8 s-tiles of 512
NT = S // 128     # 32 key tiles of 128
F32 = mybir.dt.float32
F32R = mybir.dt.float32r
BF16 = mybir.dt.bfloat16
AF = mybir.ActivationFunctionType
ALU = mybir.AluOpType

_CACHE = {}


def build_nc():
    nc = bacc.Bacc(None, target_bir_lowering=False, debug=False)
    P = lambda name, shape, dt=F32: nc.declare_dram_parameter(name, shape, dt, isOutput=False)
    xT = P("xT", [H, S], BF16)            # full x^T, bf16
    xTm = P("xTm", [H, SM])               # my columns of x^T, f32 (residual)
    wqkvT = P("wqkvT", [H, 3 * DM], BF16)  # effective (LN-folded) qkv weights
    bqkv = P("bqkv", [DM, 3])             # per-partition effective biases
    owT = P("owT", [H, H], BF16)          # o_w.T (full)
    obp = P("obp", [128, HC])             # o_b per-partition layout
    ff1T = P("ff1T", [32, 128, H], BF16)  # effective ff1, ic-major blocks
    ffb1 = P("ffb1", [128, 32])
    ff2T = P("ff2T", [I_, H], BF16)
    ffb2 = P("ffb2", [128, HC])
    y = nc.declare_dram_parameter("y", [SM, H], F32, isOutput=True)

    with tile.TileContext(nc) as tc:
        cst = tc.alloc_tile_pool(name="cst", bufs=1)
        dram = tc.alloc_tile_pool(name="dram", bufs=1, space="DRAM")
        a2a_in = [dram.tile([NC, HD + 1, SM], BF16, name=f"a2a_in{h}")
                  for h in range(2)]
        a2a_out = [dram.tile([NC, HD + 1, SM], BF16, name=f"a2a_out{h}")
                   for h in range(2)]

        # ---- constants ----
        identb = cst.tile([128, 128], BF16)
        make_identity(nc, identb)
        identf = cst.tile([128, 128], F32)
        make_identity(nc, identf)
        # E1: ones row, lhsT for broadcasting a [1,512] reciprocal row to 64
        # partitions via PE
        E1 = cst.tile([1, 128], BF16)
        nc.gpsimd.memset(E1, 1.0)

        dum = cst.tile([128, 256], BF16)
        nc.gpsimd.memset(dum, 0.0)

        bqkv_sb = cst.tile([DM, 3], F32)
        nc.sync.dma_start(out=bqkv_sb[:], in_=bqkv[:])
        obp_sb = cst.tile([128, HC], F32)
        nc.sync.dma_start(out=obp_sb[:], in_=obp[:])
        ffb1_sb = cst.tile([128, 32], F32)
        nc.sync.dma_start(out=ffb1_sb[:], in_=ffb1[:])
        ffb2_sb = cst.tile([128, HC], F32)
        nc.sync.dma_start(out=ffb2_sb[:], in_=ffb2[:])

        # ---- persistent SBUF tensors ----
        # LIFO release order: xtp (after QKV) -> atp (after attention) ->
        # ff2p/wp (end). ff2p is created early so it sits below atp/xtp on
        # the pool stack; its DMAs are issued after xtp releases.
        wp = tc.alloc_tile_pool(name="wp", bufs=1)     # weights
        ff2p = tc.alloc_tile_pool(name="ff2p", bufs=1)
        atp = tc.alloc_tile_pool(name="atp", bufs=1)   # attention state

        wqkv_t = [wp.tile([128, 3 * DM], BF16, tag=f"wqkv{hc}", name=f"wqkv{hc}") for hc in range(HC)]
        for hc in range(HC):
            nc.sync.dma_start(out=wqkv_t[hc][:], in_=wqkvT[hc * 128:(hc + 1) * 128, :])
        # attention state: both heads packed [h0 rows 0:64, h1 rows 64:128];
        # scores contract over K=64 partition windows (base 0 / 64 allowed)
        QT2 = atp.tile([128, S], BF16, tag="QT2", name="QT2")
        KT2 = atp.tile([128, S], BF16, tag="KT2", name="KT2")
        # vaug[h]: [s-part 128, 32 tiles x (64 v-dims + ones col)]
        vaug = [atp.tile([128, NT * 65], BF16, tag=f"va{h}", name=f"va{h}")
                for h in range(2)]
        for h in range(2):
            for t in range(NT):
                nc.gpsimd.memset(vaug[h][:, t * 65 + HD:t * 65 + 65], 1.0)

        # PE warmup: dependency-free matmuls so the tensor-engine clock
        # ramps while the first DMAs stream in
        with tc.tile_pool(name="ps_w", bufs=2, space="PSUM") as ps_w:
            for _ in range(40):
                psw = ps_w.tile([128, 256], F32, tag="warm")
                nc.tensor.matmul(psw[:], identb[:], dum[:], start=True, stop=True)

        # ---- Phase 1: QKV projections (LN folded into weights) ----
        with tc.tile_pool(name="p1v", bufs=3) as p1v, \
             tc.tile_pool(name="xsp", bufs=3) as xsp, \
             tc.tile_pool(name="ps_q", bufs=3, space="PSUM") as ps_q, \
             tc.tile_pool(name="ps_t", bufs=3, space="PSUM") as ps_t:
            for r in range(ST):
                rs = slice(r * 512, (r + 1) * 512)
                xr = []
                for hc in range(HC):
                    t = xsp.tile([128, 512], BF16, tag=f"x{hc}")
                    nc.sync.dma_start(out=t[:], in_=xT[hc * 128:(hc + 1) * 128, rs])
                    xr.append(t)
                for j, dest in ((0, QT2), (1, KT2)):
                    ps = ps_q.tile([128, 512], F32, tag="qkv")
                    for hc in range(HC):
                        nc.tensor.matmul(ps[:], wqkv_t[hc][:, j * 128:(j + 1) * 128],
                                         xr[hc][:], start=(hc == 0), stop=(hc == 7))
                    nc.vector.tensor_scalar_add(dest[:, rs], ps[:],
                                                bqkv_sb[:, j:j + 1])
                ps = ps_q.tile([128, 512], F32, tag="qkv")
                for hc in range(HC):
                    nc.tensor.matmul(ps[:], wqkv_t[hc][:, 2 * 128:3 * 128],
                                     xr[hc][:], start=(hc == 0), stop=(hc == 7))
                vtmp = p1v.tile([128, 512], BF16, tag="vtmp")
                nc.vector.tensor_scalar_add(vtmp[:], ps[:], bqkv_sb[:, 2:3])
                for tb in range(4):
                    t = r * 4 + tb
                    pst = ps_t.tile([128, 128], BF16, tag="vt")
                    nc.tensor.transpose(pst[:], vtmp[:, tb * 128:(tb + 1) * 128],
                                        identb[:])
                    nc.vector.tensor_copy(vaug[0][:, t * 65:t * 65 + HD], pst[:, 0:HD])
                    nc.vector.tensor_copy(vaug[1][:, t * 65:t * 65 + HD],
                                          pst[:, HD:2 * HD])
        # late-phase weights: emitted after the QKV x-stream DMAs so they
        # don't delay the first matmuls; they load during attention
        xTm_t = [wp.tile([128, SM], F32, tag=f"xTm{hc}", name=f"xTm{hc}") for hc in range(HC)]
        for hc in range(HC):
            nc.sync.dma_start(out=xTm_t[hc][:], in_=xTm[hc * 128:(hc + 1) * 128, :])
        owT_t = [wp.tile([128, H], BF16, tag=f"owT{j}", name=f"owT{j}") for j in range(HC)]
        for j in range(HC):
            nc.sync.dma_start(out=owT_t[j][:], in_=owT[j * 128:(j + 1) * 128, :])
        ff2_t = [ff2p.tile([128, H], BF16, tag=f"w2_{ic}", name=f"w2_{ic}")
                 for ic in range(32)]
        for ic in range(32):
            nc.sync.dma_start(out=ff2_t[ic][:], in_=ff2T[ic * 128:(ic + 1) * 128, :])

        # ---- Phase 2: attention (2 chains in flight, 3-wide exp) ----
        GROUPS = [tuple(range(g, min(g + 3, NT))) for g in range(0, NT, 3)]
        # h-major: all h=0 chains finish first so their AllToAll overlaps the
        # h=1 half of attention
        chains = [(h, r) for h in range(2) for r in range(ST)]
        with tc.tile_pool(name="exb", bufs=4) as exb, \
             tc.tile_pool(name="stg", bufs=4) as stg, \
             tc.tile_pool(name="ps_s", bufs=2, space="PSUM") as ps_s, \
             tc.tile_pool(name="ps_c", bufs=1, space="PSUM") as ps_c:
            for ci in range(0, len(chains), 2):
                grp = chains[ci:ci + 2]
                cps_l = [ps_c.tile([128, 512], F32, name=f"cps{ci + i}", tag=f"cps{i}")
                         for i in range(len(grp))]
                for ts in GROUPS:
                    exl = []
                    for (h, r), cps in zip(grp, cps_l):
                        rs = slice(r * 512, (r + 1) * 512)
                        n = len(ts) * 512
                        hs = slice(h * HD, (h + 1) * HD)
                        sps = ps_s.tile([128, 1536], F32, tag="sps")
                        for i, t in enumerate(ts):
                            nc.tensor.matmul(sps[:, i * 512:(i + 1) * 512],
                                             KT2[hs, t * 128:(t + 1) * 128],
                                             QT2[hs, rs], start=True, stop=True)
                        ex = exb.tile([128, 1536], BF16, tag="ex")
                        nc.scalar.activation(ex[:, 0:n], sps[:, 0:n], AF.Exp,
                                             scale=0.125)
                        exl.append(ex)
                    for (h, r), cps, ex in zip(grp, cps_l, exl):
                        for i, t in enumerate(ts):
                            nc.tensor.matmul(cps[0:HD + 1, :],
                                             vaug[h][:, t * 65:t * 65 + 65],
                                             ex[:, i * 512:(i + 1) * 512],
                                             start=(t == 0), stop=(t == NT - 1),
                                             skip_group_check=True)
                # drain unnormalized ctx + denominator to A2A buffer
                for (h, r), cps in zip(grp, cps_l):
                    st = stg.tile([HD + 1, 512], BF16, tag="st")
                    nc.vector.tensor_copy(st[:], cps[0:HD + 1, :])
                    nc.sync.dma_start(out=a2a_in[h][r, :, :], in_=st[:, :])
                if ci * 2 + len(grp) * 2 == len(chains):
                    pass  # last pair handled below
                if ci + 2 == len(chains) // 2:
                    # all h=0 chains drained: exchange them now, overlapped
                    # with the h=1 half of attention
                    nc.gpsimd.collective_compute(
                        "AllToAll", ALU.bypass,
                        replica_groups=[list(range(NC))],
                        ins=[a2a_in[0].opt()], outs=[a2a_out[0].opt()])

        atp.release()
        nc.gpsimd.collective_compute(
            "AllToAll", ALU.bypass, replica_groups=[list(range(NC))],
            ins=[a2a_in[1].opt()], outs=[a2a_out[1].opt()])

        # ---- Phase 3: normalize + out-proj (row-local) ----
        ffp = tc.alloc_tile_pool(name="ffp", bufs=1)
        x2f = [ffp.tile([128, SM], F32, tag=f"x2f{oh}", name=f"x2f{oh}")
               for oh in range(HC)]
        x2b = [ffp.tile([128, SM], BF16, tag=f"x2b{oh}", name=f"x2b{oh}")
               for oh in range(HC)]
        with tc.tile_pool(name="p3", bufs=4) as p3, \
             tc.tile_pool(name="nmp", bufs=1) as nmp, \
             tc.tile_pool(name="ps_n", bufs=2, space="PSUM") as ps_n, \
             tc.tile_pool(name="ps_o", bufs=2, space="PSUM") as ps_o:
            nm = [nmp.tile([128, SM], BF16, tag=f"nm{j}", name=f"nm{j}")
                  for j in range(NC)]

            def normalize_half(h):
                for j in range(NC):
                    cj = p3.tile([HD, SM], BF16, tag="cj")
                    nc.sync.dma_start(out=cj[:], in_=a2a_out[h][j, 0:HD, :])
                    dnbi = p3.tile([1, 512], BF16, tag="dnbi")
                    nc.sync.dma_start(out=dnbi[:], in_=a2a_out[h][j, HD:HD + 1, :])
                    # broadcast denom to 64 partitions, then approx-reciprocal
                    dps = ps_n.tile([HD, 512], F32, tag="bps")
                    nc.tensor.matmul(dps[:], E1[0:1, 0:HD], dnbi[0:1, :],
                                     start=True, stop=True)
                    rcf = p3.tile([HD, 512], F32, tag="rcf")
                    nc.vector.reciprocal_approx_fast(rcf[:], dps[:])
                    nc.vector.tensor_mul(nm[j][h * HD:(h + 1) * HD, :],
                                         cj[:], rcf[:])

            # h0 half is exchanged mid-attention; its normalize + out-proj
            # partials run while the h1 AllToAll is still in flight. Each
            # partial is a closed accumulation drained to SBUF, added back in
            # during the h1 pass.
            normalize_half(0)
            x2p0 = [ffp.tile([128, SM], F32, tag=f"x2p{oh}", name=f"x2p{oh}")
                    for oh in range(HC)]
            for oh in range(HC):
                po = ps_o.tile([128, 512], F32, tag="po")
                for j in range(NC):
                    nc.tensor.matmul(po[:], owT_t[j][0:HD, oh * 128:(oh + 1) * 128],
                                     nm[j][0:HD, :], start=(j == 0), stop=(j == 7))
                nc.vector.scalar_tensor_tensor(
                    out=x2p0[oh][:], in0=po[:], scalar=obp_sb[:, oh:oh + 1],
                    in1=xTm_t[oh][:], op0=ALU.add, op1=ALU.add)
            # keep the PE clock ramped while the second AllToAll lands
            for _ in range(176):
                psw = ps_o.tile([128, 256], F32, tag="warm")
                nc.tensor.matmul(psw[:], identb[:], dum[:], start=True, stop=True)
            normalize_half(1)
            for oh in range(HC):
                po = ps_o.tile([128, 512], F32, tag="po")
                for j in range(NC):
                    nc.tensor.matmul(po[:], owT_t[j][HD:128, oh * 128:(oh + 1) * 128],
                                     nm[j][HD:128, :], start=(j == 0), stop=(j == 7))
                nc.vector.tensor_add(x2f[oh][:], po[:], x2p0[oh][:])
                nc.scalar.activation(x2b[oh][:], x2f[oh][:], AF.Copy)

        # ---- Phase 4+5: FF (row-local, LN2 folded into ff1) ----
        with tc.tile_pool(name="hp", bufs=1) as hp, \
             tc.tile_pool(name="yp", bufs=2) as ypool, \
             tc.tile_pool(name="yrowp", bufs=1) as yrowp, \
             tc.tile_pool(name="ps_f", bufs=3, space="PSUM") as ps_f, \
             tc.tile_pool(name="ps_y", bufs=3, space="PSUM") as ps_y:
            hT = [hp.tile([128, SM], BF16, tag=f"hT{ic}", name=f"hT{ic}")
                  for ic in range(32)]
            w1p = tc.alloc_tile_pool(name="w1p", bufs=6)
            for ic in range(32):
                w1 = w1p.tile([128, H], BF16, tag="w1")
                nc.sync.dma_start(out=w1[:], in_=ff1T[ic, :, :])
                ps = ps_f.tile([128, 512], F32, tag="f1")
                for hc in range(HC):
                    nc.tensor.matmul(ps[:], w1[:, hc * 128:(hc + 1) * 128],
                                     x2b[hc][:], start=(hc == 0), stop=(hc == 7))
                nc.scalar.activation(hT[ic][:], ps[:], AF.Relu,
                                     bias=ffb1_sb[:, ic:ic + 1])
            w1p.release()
            yrow = [yrowp.tile([128, H], F32, tag=f"yr{s4}", name=f"yr{s4}")
                    for s4 in range(4)]
            for oh in range(HC):
                yps = ps_f.tile([128, 512], F32, tag="f1")
                for ic in range(32):
                    nc.tensor.matmul(yps[:], ff2_t[ic][:, oh * 128:(oh + 1) * 128],
                                     hT[ic][:], start=(ic == 0), stop=(ic == 31),
                                     skip_group_check=True)
                yf = ypool.tile([128, 512], F32, tag="yf")
                nc.vector.scalar_tensor_tensor(
                    out=yf[:], in0=yps[:], scalar=ffb2_sb[:, oh:oh + 1],
                    in1=x2f[oh][:], op0=ALU.add, op1=ALU.add)
                for s4 in range(4):
                    pst = ps_y.tile([128, 128], F32, tag="yt")
                    nc.tensor.transpose(pst[:], yf[:, s4 * 128:(s4 + 1) * 128],
                                        identf[:])
                    nc.vector.tensor_copy(yrow[s4][:, oh * 128:(oh + 1) * 128],
                                          pst[:])
            for s4 in range(4):
                nc.sync.dma_start(out=y[s4 * 128:(s4 + 1) * 128, :], in_=yrow[s4][:])

        ffp.release()
        ff2p.release()
        wp.release()
        dram.release()
        cst.release()

    nc.compile()
    return nc


def make_in_maps(inputs):
    import ml_dtypes
    bf16 = ml_dtypes.bfloat16
    f = lambda a: np.asarray(a, dtype=np.float32)
    x = f(inputs["x"])
    an_w, an_b = f(inputs["an_w"]), f(inputs["an_b"])
    fn_w, fn_b = f(inputs["fn_w"]), f(inputs["fn_b"])
    q_w, k_w, v_w = f(inputs["q_w"]), f(inputs["k_w"]), f(inputs["v_w"])
    q_b, k_b, v_b = f(inputs["q_b"]), f(inputs["k_b"]), f(inputs["v_b"])
    o_w, o_b = f(inputs["o_w"]), f(inputs["o_b"])
    ff1_w, ff1_b = f(inputs["ff1_w"]), f(inputs["ff1_b"])
    ff2_w, ff2_b = f(inputs["ff2_w"]), f(inputs["ff2_b"])

    def fold(W, w, b, bias):
        # y = W @ (diag(w)(x - mean(x)) + b) + bias  ==  Aeff @ x + c
        A = W * w[None, :]
        Aeff = A - A.mean(axis=1, keepdims=True)
        c = W @ b + bias
        return Aeff.astype(np.float32), c.astype(np.float32)

    qA, qc = fold(q_w, an_w, an_b, q_b)
    kA, kc = fold(k_w, an_w, an_b, k_b)
    vA, vc = fold(v_w, an_w, an_b, v_b)
    f1A, f1c = fold(ff1_w, fn_w, fn_b, ff1_b)

    xT_bf = np.ascontiguousarray(x.T.astype(bf16))
    xT_f = np.ascontiguousarray(x.T)
    owT = np.ascontiguousarray(o_w.T.astype(bf16))
    obp = np.ascontiguousarray(o_b.reshape(HC, 128).T)
    # [32, 128, 1024]: per inter-chunk ic, one contiguous block holding the
    # 8 h-chunk lhsT tiles side by side: [ic, p, hc*128+m] = f1A.T[hc*128+p,
    # ic*128+m]
    f1T = f1A.T  # [1024 h, 4096 inter]
    ff1T = np.ascontiguousarray(
        f1T.reshape(8, 128, 32, 128).transpose(2, 1, 0, 3).reshape(32, 128, 1024)
        .astype(bf16))
    ffb1 = np.ascontiguousarray(f1c.reshape(32, 128).T)
    ff2T = np.ascontiguousarray(ff2_w.T.astype(bf16))
    ffb2 = np.ascontiguousarray(ff2_b.reshape(HC, 128).T)

    in_maps = []
    for m in range(NC):
        dm = slice(m * DM, (m + 1) * DM)
        wqkvT = np.ascontiguousarray(
            np.concatenate([qA[dm].T, kA[dm].T, vA[dm].T], axis=1).astype(bf16))
        bqkv = np.ascontiguousarray(
            np.stack([qc[dm], kc[dm], vc[dm]], axis=1))
        in_maps.append({
            "xT": xT_bf,
            "xTm": np.ascontiguousarray(xT_f[:, m * SM:(m + 1) * SM]),
            "wqkvT": wqkvT,
            "bqkv": bqkv,
            "owT": owT,
            "obp": obp,
            "ff1T": ff1T,
            "ffb1": ffb1,
            "ff2T": ff2T,
            "ffb2": ffb2,
        })
    return in_maps


def kernel(**inputs) -> np.ndarray:
    from concourse.bass_utils import run_bass_kernel_spmd
    if "nc" not in _CACHE:
        _CACHE["nc"] = build_nc()
    nc = _CACHE["nc"]
    in_maps = make_in_maps(inputs)
    res = run_bass_kernel_spmd(nc, in_maps, core_ids=list(range(NC)))
    return np.concatenate([res.results[m]["y"] for m in range(NC)], axis=0)
